# revision 1
# baseline (speedup 1.0000x reference)
"""GQA multi-head attention (B=2, S=2048, D=2048, 32 q-heads / 8 kv-heads)
on 8 Trainium2 NeuronCores.

Sharding: tensor-parallel over kv-head groups. Core c owns kv head c and its
4 query heads: Wq column-shard [2048, 256], Wk/Wv column-shard [2048, 64],
Wo row-shard [256, 2048]. Each core computes a full-shape partial output
(its heads' contribution through Wo); the host sums the 8 partials.

Per-core dataflow (all matmuls bf16 operands, fp32 PSUM accumulate):
  QT  [256, B*S] = Wq_c^T @ q^T          (q^T fed from host)
  KT  [64,  B*S] = Wk_c^T @ k^T          (duplicated to partitions 0:64 and
                                          64:128 so even/odd head scores can
                                          row-pack and run concurrently on PE)
  V   [B*S, 64]  = v rows @ Wv_c         (natural layout; +ones column)
  S^T [k, q] = KT-tile.T @ QT            (scores transposed: softmax axis on
                                          partitions -> no P transpose needed)
  expS^T = exp(S^T * 1/8)                (ACT, scale folded in; no max-sub:
                                          |scores/8| < ~6 so fp32 exp is safe)
  ctx^T+rowsum [65, q] = V_aug^T @ expS^T (ones column gives softmax denom)
  ctx_norm = ctx^T * (1/rowsum)          (DVE reciprocal into partition 0 +
                                          GPSIMD partition_broadcast)
  out_partial [B*S, 2048] = ctx_norm-tiles.T @ Wo_c
"""

from contextlib import ExitStack

import numpy as np
import ml_dtypes

import jax

try:
    jax.config.update("jax_compilation_cache_dir", "/tmp/jax_bass_cache")
    jax.config.update("jax_persistent_cache_min_compile_time_secs", 1.0)
except Exception:
    pass

from jax.sharding import Mesh, PartitionSpec, NamedSharding
from jax.experimental.shard_map import shard_map

import concourse.bass as bass
import concourse.mybir as mybir
import concourse.tile as tile
from concourse import bacc, bass2jax

BF16 = mybir.dt.bfloat16
F32 = mybir.dt.float32
AF = mybir.ActivationFunctionType

B, S, DM = 2, 2048, 2048
HKV, G, DH = 8, 4, 64
DQ = G * DH            # 256: per-core q-projection width
NC = 8
DT = DM // 128         # 16 contraction tiles
BS = B * S             # 4096
SCALE = 1.0 / 8.0      # 1/sqrt(64)

_cache = {}

import os as _os
PHASES = _os.environ.get("K_PHASES", "ABC")


def _emit(ctx, tc, qT, kT, vT, wq, wk, wv, wo, out):
    nc = tc.nc

    pp = ctx.enter_context(tc.tile_pool(name="persist", bufs=1))
    wq_sb = pp.tile([128, DT, DQ], BF16, tag="wq")
    wk_sb = pp.tile([128, DT, DH], BF16, tag="wk")
    wv_sb = pp.tile([128, DT, DH], BF16, tag="wv")
    wo_sb = pp.tile([128, 2, DM], BF16, tag="wo")
    qtp = pp.tile([128, 2, BS], BF16, tag="qtp")    # QT pairs [p, hp, b*S+s]
    ktd = pp.tile([128, BS], BF16, tag="ktd")       # KT duplicated both halves
    vsb = pp.tile([128, BS // 128, DH + 1], BF16, tag="vsb")  # V + ones col
    ctxT = pp.tile([128, 2, BS], BF16, tag="ctxT")  # normalized ctx^T pairs
    ident = pp.tile([DH, DH], BF16, tag="ident")

    from concourse.masks import make_identity
    make_identity(nc, ident[:])
    nc.sync.dma_start(wq_sb[:], wq.rearrange("(dt p) m -> p dt m", p=128))
    nc.sync.dma_start(wk_sb[:], wk.rearrange("(dt p) m -> p dt m", p=128))
    nc.sync.dma_start(wv_sb[:], wv.rearrange("(dt p) m -> p dt m", p=128))
    nc.sync.dma_start(wo_sb[:], wo.rearrange("(i p) d -> p i d", p=128))
    nc.gpsimd.memset(vsb[:, :, DH], 1.0)

    stage = ctx.enter_context(tc.tile_pool(name="stage", bufs=2))
    expp = ctx.enter_context(tc.tile_pool(name="expp", bufs=4))
    smal = ctx.enter_context(tc.tile_pool(name="small", bufs=2))
    outp = ctx.enter_context(tc.tile_pool(name="outp", bufs=2))
    psum = ctx.enter_context(tc.tile_pool(name="psum", bufs=1, space="PSUM"))

    # ---------------- Phase A: projections ----------------
    for b in range(B if "A" in PHASES else 0):
        bo = b * S
        for qc in range(4):
            so = qc * 512
            k_ch = stage.tile([128, DT, 512], BF16, tag="instage", bufs=3,
                              name=f"k_ch_{b}_{qc}")
            nc.sync.dma_start(
                k_ch[:],
                kT[b].rearrange("(dt p) s -> p dt s", p=128)[:, :, so:so + 512])
            v_ch = stage.tile([128, DT, 512], BF16, tag="instage", bufs=3,
                              name=f"v_ch_{b}_{qc}")
            nc.sync.dma_start(
                v_ch[:],
                vT[b].rearrange("(dt p) s -> p dt s", p=128)[:, :, so:so + 512])
            # K^T and V^T projections col-packed: K lands on array columns
            # 0:64 (psum rows 0:64), V^T on columns 64:128 — the two streams
            # run concurrently through disjoint column groups.
            pk = psum.tile([128, 512], F32, tag="kv", bufs=1,
                           name=f"pk_{b}_{qc}")
            for dt in range(DT):
                nc.tensor.matmul(
                    pk[0:DH, :], wk_sb[:, dt, :], k_ch[:, dt, :],
                    start=(dt == 0), stop=(dt == DT - 1))
                nc.tensor.matmul(
                    pk[DH:128, :], wv_sb[:, dt, :], v_ch[:, dt, :],
                    start=(dt == 0), stop=(dt == DT - 1))
            nc.vector.tensor_copy(ktd[0:DH, bo + so:bo + so + 512], pk[0:DH, :])
            nc.vector.tensor_copy(ktd[DH:128, bo + so:bo + so + 512], pk[0:DH, :])
            vt_sb = smal.tile([DH, 512], BF16, tag="vt", name=f"vt_{b}_{qc}")
            nc.vector.tensor_copy(vt_sb[:], pk[DH:128, :])
            for ss in range(4):
                pv = psum.tile([128, DH], BF16, tag="kv", bufs=1,
                               name=f"pv_{b}_{qc}_{ss}")
                nc.tensor.transpose(
                    pv[:], vt_sb[:, ss * 128:(ss + 1) * 128], ident[:])
                nc.vector.tensor_copy(
                    vsb[:, b * 16 + qc * 4 + ss, 0:DH], pv[:])

        for qc in range(4):
            so = qc * 512
            q_ch = stage.tile([128, DT, 512], BF16, tag="instage", bufs=3,
                              name=f"q_ch_{b}_{qc}")
            nc.sync.dma_start(
                q_ch[:],
                qT[b].rearrange("(dt p) s -> p dt s", p=128)[:, :, so:so + 512])
            for m in range(2):
                pq = psum.tile([128, 512], F32, tag="mm", bufs=3,
                               name=f"pq_{b}_{qc}_{m}")
                for dt in range(DT):
                    nc.tensor.matmul(
                        pq[:], wq_sb[:, dt, m * 128:(m + 1) * 128],
                        q_ch[:, dt, :], start=(dt == 0), stop=(dt == DT - 1))
                nc.vector.tensor_copy(qtp[:, m, bo + so:bo + so + 512], pq[:])

    # --- Phases B+C interleaved per batch: C(b) emitted right after B(b) ---
    def phase_b(b):
        bo = b * S
        for hp in range(2):
            for qc in range(4):
                qoff = bo + qc * 512
                exps = [expp.tile([128, DT, 512], BF16, tag="exp", bufs=3,
                                  name=f"exp{j}_{b}_{hp}_{qc}")
                        for j in range(2)]
                for kt2 in range(DT // 2):
                    pss = [psum.tile([128, 2, 512], F32, tag="sc", bufs=2,
                                     name=f"pss{j}_{b}_{hp}_{qc}_{kt2}")
                           for j in range(2)]
                    for j2 in range(2):
                        kt = 2 * kt2 + j2
                        koff = bo + kt * 128
                        for j in range(2):  # head j: rows j*64:(j+1)*64
                            lo, hi = j * DH, (j + 1) * DH
                            nc.tensor.matmul(
                                pss[j][:, j2, :],
                                ktd[lo:hi, koff:koff + 128],
                                qtp[lo:hi, hp, qoff:qoff + 512])
                    for j in range(2):
                        nc.scalar.activation(
                            exps[j][:, 2 * kt2:2 * kt2 + 2, :], pss[j][:],
                            AF.Exp, scale=SCALE)
                for j in range(2):
                    pc = psum.tile([128, 512], F32, tag="mm", bufs=3,
                                   name=f"pc{j}_{b}_{hp}_{qc}")
                    for kt in range(DT):
                        nc.tensor.matmul(
                            pc[0:DH + 1, :], vsb[:, b * 16 + kt, :],
                            exps[j][:, kt, :],
                            start=(kt == 0), stop=(kt == DT - 1))
                    rc = smal.tile([128, 512], F32, tag="recip",
                                   name=f"rc{j}_{b}_{hp}_{qc}")
                    nc.vector.reciprocal(rc[0:1, :], pc[DH:DH + 1, :])
                    bc_sb = smal.tile([DH, 512], F32, tag="bc_sb",
                                      name=f"bc{j}_{b}_{hp}_{qc}")
                    nc.gpsimd.partition_broadcast(bc_sb[:], rc[0:1, :])
                    nc.vector.tensor_mul(
                        ctxT[j * DH:(j + 1) * DH, hp, qoff:qoff + 512],
                        pc[0:DH, :], bc_sb[:])

    def phase_c(st):
        ost = outp.tile([128, DM], BF16, tag="ostage", bufs=4, name=f"ost_{st}")
        for ch in range(4):
            po = psum.tile([128, 512], F32, tag="mm", bufs=3,
                           name=f"po_{st}_{ch}")
            for i in range(2):
                nc.tensor.matmul(
                    po[:], ctxT[:, i, st * 128:(st + 1) * 128],
                    wo_sb[:, i, ch * 512:(ch + 1) * 512],
                    start=(i == 0), stop=(i == 1))
            dst = ost[:, ch * 512:(ch + 1) * 512]
            if ch % 2 == 0:
                nc.vector.tensor_copy(dst, po[:])
            else:
                nc.scalar.copy(dst, po[:])
        nc.sync.dma_start(out[st * 128:(st + 1) * 128, :], ost[:])

    for b in range(B if "B" in PHASES else 0):
        phase_b(b)
        if "C" in PHASES:
            for st in range(b * 16, (b + 1) * 16):
                phase_c(st)


def _build():
    nc = bacc.Bacc("TRN2", target_bir_lowering=False, debug=False, num_devices=NC)
    qT = nc.dram_tensor("qT", [B, DM, S], BF16, kind="ExternalInput")
    kT = nc.dram_tensor("kT", [B, DM, S], BF16, kind="ExternalInput")
    vT = nc.dram_tensor("vT", [B, DM, S], BF16, kind="ExternalInput")
    wq = nc.dram_tensor("wq", [DM, DQ], BF16, kind="ExternalInput")
    wk = nc.dram_tensor("wk", [DM, DH], BF16, kind="ExternalInput")
    wv = nc.dram_tensor("wv", [DM, DH], BF16, kind="ExternalInput")
    wo = nc.dram_tensor("wo", [DQ, DM], BF16, kind="ExternalInput")
    out = nc.dram_tensor("out", [BS, DM], BF16, kind="ExternalOutput")
    with tile.TileContext(nc) as tc:
        with ExitStack() as ctx:
            _emit(ctx, tc, qT.ap(), kT.ap(), vT.ap(), wq.ap(), wk.ap(),
                  wv.ap(), wo.ap(), out.ap())
    nc.compile()
    return nc


def _make_runner(nc, n_cores=NC):
    """Build the sharded jit callable once; reuse across kernel() calls."""
    bass2jax.install_neuronx_cc_hook()
    partition_name = nc.partition_id_tensor.name if nc.partition_id_tensor else None
    in_names, out_names, out_avals, zero_outs = [], [], [], []
    for alloc in nc.m.functions[0].allocations:
        if not isinstance(alloc, mybir.MemoryLocationSet):
            continue
        name = alloc.memorylocations[0].name
        if alloc.kind == "ExternalInput":
            if name != partition_name:
                in_names.append(name)
        elif alloc.kind == "ExternalOutput":
            out_names.append(name)
            shape = tuple(alloc.tensor_shape)
            dtype = mybir.dt.np(alloc.dtype)
            out_avals.append(jax.core.ShapedArray(shape, dtype))
            zero_outs.append(np.zeros(shape, dtype))
    n_params = len(in_names)
    n_outs = len(out_avals)
    in_names_all = in_names + out_names
    if partition_name is not None:
        in_names_all.append(partition_name)
    donate = tuple(range(n_params, n_params + n_outs))

    def _body(*args):
        operands = list(args)
        if partition_name is not None:
            operands.append(bass2jax.partition_id_tensor())
        outs = bass2jax._bass_exec_p.bind(
            *operands,
            out_avals=tuple(out_avals),
            in_names=tuple(in_names_all),
            out_names=tuple(out_names),
            lowering_input_output_aliases=(),
            sim_require_finite=True,
            sim_require_nnan=True,
            nc=nc,
        )
        return tuple(outs)

    devices = jax.devices()[:n_cores]
    mesh = Mesh(np.asarray(devices), ("core",))
    in_specs = (PartitionSpec("core"),) * (n_params + n_outs)
    out_specs = (PartitionSpec("core"),) * len(out_names)
    sharded = jax.jit(
        shard_map(_body, mesh=mesh, in_specs=in_specs, out_specs=out_specs,
                  check_rep=False),
        donate_argnums=donate, keep_unused=True)
    sh = NamedSharding(mesh, PartitionSpec("core"))
    return sharded, in_names, out_names, zero_outs, sh


def _run(in_maps):
    if "nc" not in _cache:
        _cache["nc"] = _build()
    if "runner" not in _cache:
        _cache["runner"] = _make_runner(_cache["nc"])
    sharded, in_names, out_names, zero_outs, sh = _cache["runner"]
    n = NC
    concat_in = [
        jax.device_put(
            np.concatenate([np.asarray(in_maps[c][nm]) for c in range(n)], 0), sh)
        for nm in in_names
    ]
    zeros = [
        jax.device_put(np.zeros((n * z.shape[0], *z.shape[1:]), z.dtype), sh)
        for z in zero_outs
    ]
    outs = sharded(*concat_in, *zeros)
    i = out_names.index("out")
    arr = np.asarray(outs[i])           # [NC*BS, DM]
    return arr.reshape(n, BS, DM)


def kernel(q, k, v, Wq, Wk, Wv, Wo):
    q = np.asarray(q, dtype=np.float32)
    k = np.asarray(k, dtype=np.float32)
    v = np.asarray(v, dtype=np.float32)
    bf = ml_dtypes.bfloat16
    qTh = np.ascontiguousarray(q.astype(bf).transpose(0, 2, 1))
    kTh = np.ascontiguousarray(k.astype(bf).transpose(0, 2, 1))
    vTh = np.ascontiguousarray(v.astype(bf).transpose(0, 2, 1))
    Wqb = np.asarray(Wq, dtype=np.float32).astype(bf)
    Wkb = np.asarray(Wk, dtype=np.float32).astype(bf)
    Wvb = np.asarray(Wv, dtype=np.float32).astype(bf)
    Wob = np.asarray(Wo, dtype=np.float32).astype(bf)

    in_maps = []
    for c in range(NC):
        in_maps.append({
            "qT": qTh, "kT": kTh, "vT": vTh,
            "wq": np.ascontiguousarray(Wqb[:, c * DQ:(c + 1) * DQ]),
            "wk": np.ascontiguousarray(Wkb[:, c * DH:(c + 1) * DH]),
            "wv": np.ascontiguousarray(Wvb[:, c * DH:(c + 1) * DH]),
            "wo": np.ascontiguousarray(Wob[c * DQ:(c + 1) * DQ, :]),
        })
    partials = _run(in_maps)
    out = partials.astype(np.float32, copy=False).sum(axis=0)
    return out.reshape(B, S, DM)



# revision 8
# speedup vs baseline: 1.0930x; 1.0930x over previous
"""GQA multi-head attention (B=2, S=2048, D=2048, 32 q-heads / 8 kv-heads)
on 8 Trainium2 NeuronCores.

Sharding: tensor-parallel over kv-head groups. Core c owns kv head c and its
4 query heads: Wq column-shard [2048, 256], Wk/Wv column-shard [2048, 64],
Wo row-shard [256, 2048]. Each core computes a full-shape partial output
(its heads' contribution through Wo); the host sums the 8 partials.

Per-core dataflow (all matmuls bf16 operands, fp32 PSUM accumulate). PE cost
on TRN2 is charged per output-free-size row, so every matmul is oriented to
keep the output free dim minimal for the math it does:
  QT  [256, B*S] = Wq_c^T @ q^T     (q^T fed from host; psum [128, 512])
  K   [keys, 64] = kT-tile^T @ Wk_c (input tile stationary: psum [128keys,64],
                                     64-row charge; K^T recovered via DMA XBAR
                                     transpose into ktd, dup'd to both halves)
  V   [keys, 64] = vT-tile^T @ Wv_c (same flip; lands directly in vsb layout)
  S^T [k, q] = KT-tile.T @ QT       (scores transposed: softmax axis on
                                     partitions)
  expS^T = exp(S^T * 1/8)           (ACT, scale folded in; no max-sub:
                                     |scores/8| small so fp32 exp is safe)
  ctx [q, 65] = expS^T-tile.T @ V_aug  (exp tile stationary: 65-row charge;
                                     ones column gives softmax denominator in
                                     free col 64 -> normalization is a fused
                                     per-partition DVE divide in the required
                                     psum->sbuf copy)
  ctx^T via DMA XBAR transpose into ctxT [dq, q]
  out_partial [B*S, 2048] = ctxT-tiles.T @ Wo_c  (psum copies split DVE/Pool)

Schedule: ctx lags scores by one head-step so PE's in-order stream never
waits on ACT exp; batch-1 projections and out-proj groups are interleaved
into batch-0's exp-bound window.
"""
from collections import deque

from contextlib import ExitStack

import numpy as np
import ml_dtypes

import jax

try:
    jax.config.update("jax_compilation_cache_dir", "/tmp/jax_bass_cache")
    jax.config.update("jax_persistent_cache_min_compile_time_secs", 1.0)
except Exception:
    pass

from jax.sharding import Mesh, PartitionSpec, NamedSharding
from jax.experimental.shard_map import shard_map

import concourse.bass as bass
import concourse.mybir as mybir
import concourse.tile as tile
from concourse import bacc, bass2jax

BF16 = mybir.dt.bfloat16
F32 = mybir.dt.float32
AF = mybir.ActivationFunctionType

B, S, DM = 2, 2048, 2048
HKV, G, DH = 8, 4, 64
DQ = G * DH            # 256: per-core q-projection width
NC = 8
DT = DM // 128         # 16 contraction tiles
BS = B * S             # 4096
SCALE = 1.0 / 8.0      # 1/sqrt(64)

_cache = {}

import os as _os
# "pe" | "dma": the DMA XBAR route is numerically correct in CoreSim but
# races on the compiled NEFF path, so PE transposes are the default.
TRANS = _os.environ.get("K_TRANS", "pe")


def _emit(ctx, tc, qT, kT, vT, wq, wk, wv, wo, out):
    nc = tc.nc

    pp = ctx.enter_context(tc.tile_pool(name="persist", bufs=1))
    wq_sb = pp.tile([128, DT, DQ], BF16, tag="wq")
    wk_sb = pp.tile([128, DT, DH], BF16, tag="wk")
    wv_sb = pp.tile([128, DT, DH], BF16, tag="wv")
    wo_sb = pp.tile([128, 2, DM], BF16, tag="wo")
    qtp = pp.tile([128, 2, BS], BF16, tag="qtp")    # QT pairs [p, hp, b*S+s]
    ktd = pp.tile([128, BS], BF16, tag="ktd")       # KT duplicated both halves
    vsb = pp.tile([128, BS // 128, DH + 1], BF16, tag="vsb")  # V + ones col
    ctxT = pp.tile([128, 2, BS], BF16, tag="ctxT")  # normalized ctx^T pairs

    if TRANS == "pe":
        ident = pp.tile([128, 128], BF16, tag="ident")
        from concourse.masks import make_identity
        make_identity(nc, ident[:])

    # Weight DMA issue order matters: DMA sem-waits hold the SP sequencer,
    # and the lead-in is gated by K/V arrival. wk/wv/wq first (tiny), K/V
    # chunks next, wo deferred past the first q chunk.
    nc.sync.dma_start(wk_sb[:], wk.rearrange("(dt p) m -> p dt m", p=128))
    nc.sync.dma_start(wv_sb[:], wv.rearrange("(dt p) m -> p dt m", p=128))
    nc.sync.dma_start(wq_sb[:], wq.rearrange("(dt p) m -> p dt m", p=128))
    nc.gpsimd.memset(vsb[:, :, DH], 1.0)

    stage = ctx.enter_context(tc.tile_pool(name="stage", bufs=2))
    expp = ctx.enter_context(tc.tile_pool(name="expp", bufs=2))
    smal = ctx.enter_context(tc.tile_pool(name="small", bufs=2))
    outp = ctx.enter_context(tc.tile_pool(name="outp", bufs=2))
    psum = ctx.enter_context(tc.tile_pool(name="psum", bufs=1, space="PSUM"))

    DIV = mybir.AluOpType.divide

    # ---- work-piece pump: ~1-2.5us PE pieces round-robined between the
    # scores pieces of each slot so the 4-deep ACT wait queue never starves.
    work = deque()

    def pump_one():
        while work:
            try:
                next(work[0])
                return True
            except StopIteration:
                work.popleft()
        return False

    def run_gen(g):
        for _ in g:
            pass

    def advance(g):
        return lambda: next(g, None)

    # ---- Phase A emitters -------------------------------------------------
    def a_kv(b, qc):
        """K/V projections, input-tile-stationary: psum [128 keys, 64].
        DMA issue is eager (at call time); compute comes as gen pieces."""
        bo, so = b * S, qc * 512
        k_ch = stage.tile([128, DT, 512], BF16, tag="kv_st", bufs=3,
                          name=f"kch_{b}_{qc}")
        nc.sync.dma_start(
            k_ch[:],
            kT[b].rearrange("(dt p) s -> p dt s", p=128)[:, :, so:so + 512])
        v_ch = stage.tile([128, DT, 512], BF16, tag="kv_st", bufs=3,
                          name=f"vch_{b}_{qc}")
        nc.sync.dma_start(
            v_ch[:],
            vT[b].rearrange("(dt p) s -> p dt s", p=128)[:, :, so:so + 512])

        def gen():
            kv = psum.tile([128, 4, 2, DH], F32, tag="pa", bufs=2,
                           name=f"kv_{b}_{qc}")
            for ki in range(4):
                for dt in range(DT):
                    nc.tensor.matmul(
                        kv[:, ki, 0, :], k_ch[:, dt, ki * 128:(ki + 1) * 128],
                        wk_sb[:, dt, :], start=(dt == 0), stop=(dt == DT - 1))
                for dt in range(DT):
                    nc.tensor.matmul(
                        kv[:, ki, 1, :], v_ch[:, dt, ki * 128:(ki + 1) * 128],
                        wv_sb[:, dt, :], start=(dt == 0), stop=(dt == DT - 1))
                yield
            nc.vector.tensor_copy(
                vsb[:, b * 16 + qc * 4:b * 16 + qc * 4 + 4, 0:DH],
                kv[:, :, 1, :])
            ktmp = smal.tile([128, 4, DH], BF16, tag="ktmp", bufs=2,
                             name=f"ktmp_{b}_{qc}")
            nc.vector.tensor_copy(ktmp[:], kv[:, :, 0, :])
            yield
            if TRANS == "dma":
                for pr in range(2):
                    src = ktmp[:, 2 * pr:2 * pr + 2, :]
                    for half in range(2):
                        dst = ktd[DH * half:DH * half + DH,
                                  bo + so + pr * 256:bo + so + (pr + 1) * 256]
                        nc.sync.dma_start(
                            dst.rearrange("p (t k) -> p t k", t=2), src,
                            transpose=True)
            else:
                for ki in range(4):
                    koff = bo + so + ki * 128
                    ktp = psum.tile([128, 128], BF16, tag="pc", bufs=2,
                                    name=f"ktp_{b}_{qc}_{ki}")
                    for half in range(2):
                        nc.tensor.transpose(
                            ktp[DH * half:DH * half + DH, :], ktmp[:, ki, :],
                            ident[:], tile_position=(0, DH * half))
                    nc.vector.tensor_copy(ktd[:, koff:koff + 128], ktp[:])
            yield

        return gen()

    def a_q_dma(b, qc):
        bo, so = b * S, qc * 512
        q_ch = stage.tile([128, DT, 512], BF16, tag="q_st", bufs=2,
                          name=f"qch_{b}_{qc}")
        nc.sync.dma_start(
            q_ch[:],
            qT[b].rearrange("(dt p) s -> p dt s", p=128)[:, :, so:so + 512])
        return q_ch

    def a_q_proj(b, qc, q_ch):
        bo, so = b * S, qc * 512
        for m in range(2):
            pq = psum.tile([128, 512], F32, tag="pa", bufs=2,
                           name=f"pq_{b}_{qc}_{m}")
            for dt in range(DT):
                nc.tensor.matmul(
                    pq[:], wq_sb[:, dt, m * 128:(m + 1) * 128],
                    q_ch[:, dt, :], start=(dt == 0), stop=(dt == DT - 1))
            nc.vector.tensor_copy(qtp[:, m, bo + so:bo + so + 512], pq[:])
            yield

    # ---- Phase C piece ----------------------------------------------------
    def c_st(b, qc, qt):
        st = b * 16 + qc * 4 + qt
        ost = outp.tile([128, DM], BF16, tag="ost", bufs=3, name=f"ost_{st}")
        for ch in range(4):
            po = psum.tile([128, 512], F32, tag="pa", bufs=2,
                           name=f"po_{st}_{ch}")
            for i in range(2):
                nc.tensor.matmul(
                    po[:], ctxT[:, i, st * 128:(st + 1) * 128],
                    wo_sb[:, i, ch * 512:(ch + 1) * 512],
                    start=(i == 0), stop=(i == 1))
            nc.vector.tensor_copy(ost[:, ch * 512:(ch + 1) * 512], po[:])
        nc.sync.dma_start(out[st * 128:(st + 1) * 128, :], ost[:])

    # ---- Phase B emitters -------------------------------------------------
    cn_map = {}

    def ctx_gen(b, qc, h, ex):
        """ctx [q, 65] with exp tile stationary; normalization fused into the
        required psum->sbuf copy; per-qt transposes + out-proj pieces."""
        i, j = h // 2, h % 2
        if j == 0:
            cn_map[(b, qc, i)] = [
                smal.tile([128, 2, DH], BF16, tag="cn", bufs=10,
                          name=f"cn_{b}_{qc}_{i}_{qt}") for qt in range(4)]
        cn = cn_map[(b, qc, i)]
        pcx = psum.tile([128, 4, DH + 1], F32, tag="pc", bufs=2,
                        name=f"pcx_{b}_{qc}_{h}")
        for qt2 in range(2):
            for qt in (2 * qt2, 2 * qt2 + 1):
                for kt in range(DT):
                    nc.tensor.matmul(
                        pcx[:, qt, :], ex[:, kt, qt * 128:(qt + 1) * 128],
                        vsb[:, b * 16 + kt, :],
                        start=(kt == 0), stop=(kt == DT - 1))
            yield
        rr = smal.tile([128, 4], F32, tag="rr", bufs=3, name=f"rr_{b}_{qc}_{h}")
        nc.vector.reciprocal(rr[:], pcx[:, :, DH])
        for qt in range(4):
            nc.vector.tensor_scalar_mul(
                cn[qt][:, j, :], pcx[:, qt, 0:DH], rr[:, qt:qt + 1])
        yield
        if j == 1:
            qoff = b * S + qc * 512
            for qt in range(4):
                dst = ctxT[:, i, qoff + qt * 128:qoff + (qt + 1) * 128]
                if TRANS == "dma":
                    nc.sync.dma_start(dst, cn[qt][:], transpose=True)
                else:
                    ctp = psum.tile([128, 128], BF16, tag="pc", bufs=2,
                                    name=f"ctp_{b}_{qc}_{i}_{qt}")
                    nc.tensor.transpose(ctp[:], cn[qt][:], ident[:])
                    nc.vector.tensor_copy(dst, ctp[:])
                c_st(b, qc, qt)
                yield
            del cn_map[(b, qc, i)]

    def scores_slot(b, qc, h, actions):
        """Emit the 8 score/exp pieces of slot (qc, h), interleaving one
        action (forced work or deque pump) after each piece."""
        m, j = h // 2, h % 2
        bo = b * S
        qoff = bo + qc * 512
        ex = expp.tile([128, DT, 512], BF16, tag="exp", bufs=2,
                       name=f"ex_{b}_{qc}_{h}")
        for kt2 in range(DT // 2):
            pss = psum.tile([128, 2, 512], F32, tag="sc", bufs=2,
                            name=f"pss_{b}_{qc}_{h}_{kt2}")
            for t in range(2):
                koff = bo + (2 * kt2 + t) * 128
                nc.tensor.matmul(
                    pss[:, t, :], ktd[j * DH:(j + 1) * DH, koff:koff + 128],
                    qtp[j * DH:(j + 1) * DH, m, qoff:qoff + 512])
            nc.scalar.activation(
                ex[:, 2 * kt2:2 * kt2 + 2, :], pss[:], AF.Exp, scale=SCALE)
            if kt2 < len(actions):
                actions[kt2]()
            else:
                pump_one()
        for act in actions[DT // 2:]:
            act()
        return ex

    # ---- Schedule ---------------------------------------------------------
    # Lead-in: K/V chunks stream while projections chase; first q chunk's
    # DMA is slotted before the last K/V pair so q-proj overlaps the tail.
    run_gen(a_kv(0, 0))
    run_gen(a_kv(0, 1))
    run_gen(a_kv(0, 2))
    g_last = a_kv(0, 3)            # issues k3/v3 DMA
    q_ch0 = a_q_dma(0, 0)
    run_gen(g_last)
    run_gen(a_q_proj(0, 0, q_ch0))
    nc.sync.dma_start(wo_sb[:], wo.rearrange("(i p) d -> p i d", p=128))

    q_chunks = {}
    q_projs = {}

    for b in range(B):
        pend = deque()
        for qc in range(4):
            for h in range(4):
                actions = []
                if h == 0 and qc < 3:
                    # q chunk (b, qc+1): DMA now, proj pieces forced next slot
                    q_chunks[(b, qc + 1)] = a_q_dma(b, qc + 1)
                    g = a_q_proj(b, qc + 1, q_chunks[(b, qc + 1)])
                    q_projs[(b, qc + 1)] = g
                    actions += [advance(g), advance(g)]
                if b == 0 and h == 1:
                    g = a_kv(1, qc)
                    actions += [advance(g)] * 6
                if b == 0 and qc == 3 and h == 2:
                    q_chunks[(1, 0)] = a_q_dma(1, 0)
                if pend:
                    work.append(ctx_gen(b, *pend.popleft()))
                ex = scores_slot(b, qc, h, actions)
                pend.append((qc, h, ex))
        work.append(ctx_gen(b, *pend.popleft()))
        if b == 0:
            run_gen(a_q_proj(1, 0, q_chunks[(1, 0)]))
        while pump_one():
            pass


def _build():
    nc = bacc.Bacc("TRN2", target_bir_lowering=False, debug=False, num_devices=NC)
    qT = nc.dram_tensor("qT", [B, DM, S], BF16, kind="ExternalInput")
    kT = nc.dram_tensor("kT", [B, DM, S], BF16, kind="ExternalInput")
    vT = nc.dram_tensor("vT", [B, DM, S], BF16, kind="ExternalInput")
    wq = nc.dram_tensor("wq", [DM, DQ], BF16, kind="ExternalInput")
    wk = nc.dram_tensor("wk", [DM, DH], BF16, kind="ExternalInput")
    wv = nc.dram_tensor("wv", [DM, DH], BF16, kind="ExternalInput")
    wo = nc.dram_tensor("wo", [DQ, DM], BF16, kind="ExternalInput")
    out = nc.dram_tensor("out", [BS, DM], BF16, kind="ExternalOutput")
    with tile.TileContext(nc) as tc:
        with ExitStack() as ctx:
            _emit(ctx, tc, qT.ap(), kT.ap(), vT.ap(), wq.ap(), wk.ap(),
                  wv.ap(), wo.ap(), out.ap())
    nc.compile()
    return nc


def _make_runner(nc, n_cores=NC):
    """Build the sharded jit callable once; reuse across kernel() calls."""
    bass2jax.install_neuronx_cc_hook()
    partition_name = nc.partition_id_tensor.name if nc.partition_id_tensor else None
    in_names, out_names, out_avals, zero_outs = [], [], [], []
    for alloc in nc.m.functions[0].allocations:
        if not isinstance(alloc, mybir.MemoryLocationSet):
            continue
        name = alloc.memorylocations[0].name
        if alloc.kind == "ExternalInput":
            if name != partition_name:
                in_names.append(name)
        elif alloc.kind == "ExternalOutput":
            out_names.append(name)
            shape = tuple(alloc.tensor_shape)
            dtype = mybir.dt.np(alloc.dtype)
            out_avals.append(jax.core.ShapedArray(shape, dtype))
            zero_outs.append(np.zeros(shape, dtype))
    n_params = len(in_names)
    n_outs = len(out_avals)
    in_names_all = in_names + out_names
    if partition_name is not None:
        in_names_all.append(partition_name)
    donate = tuple(range(n_params, n_params + n_outs))

    def _body(*args):
        operands = list(args)
        if partition_name is not None:
            operands.append(bass2jax.partition_id_tensor())
        outs = bass2jax._bass_exec_p.bind(
            *operands,
            out_avals=tuple(out_avals),
            in_names=tuple(in_names_all),
            out_names=tuple(out_names),
            lowering_input_output_aliases=(),
            sim_require_finite=True,
            sim_require_nnan=True,
            nc=nc,
        )
        return tuple(outs)

    devices = jax.devices()[:n_cores]
    mesh = Mesh(np.asarray(devices), ("core",))
    in_specs = (PartitionSpec("core"),) * (n_params + n_outs)
    out_specs = (PartitionSpec("core"),) * len(out_names)
    sharded = jax.jit(
        shard_map(_body, mesh=mesh, in_specs=in_specs, out_specs=out_specs,
                  check_rep=False),
        donate_argnums=donate, keep_unused=True)
    sh = NamedSharding(mesh, PartitionSpec("core"))
    return sharded, in_names, out_names, zero_outs, sh


def _run(in_maps):
    if "nc" not in _cache:
        _cache["nc"] = _build()
    if "runner" not in _cache:
        _cache["runner"] = _make_runner(_cache["nc"])
    sharded, in_names, out_names, zero_outs, sh = _cache["runner"]
    n = NC
    concat_in = [
        jax.device_put(
            np.concatenate([np.asarray(in_maps[c][nm]) for c in range(n)], 0), sh)
        for nm in in_names
    ]
    zeros = [
        jax.device_put(np.zeros((n * z.shape[0], *z.shape[1:]), z.dtype), sh)
        for z in zero_outs
    ]
    outs = sharded(*concat_in, *zeros)
    i = out_names.index("out")
    arr = np.asarray(outs[i])           # [NC*BS, DM]
    return arr.reshape(n, BS, DM)


def kernel(q, k, v, Wq, Wk, Wv, Wo):
    q = np.asarray(q, dtype=np.float32)
    k = np.asarray(k, dtype=np.float32)
    v = np.asarray(v, dtype=np.float32)
    bf = ml_dtypes.bfloat16
    qTh = np.ascontiguousarray(q.astype(bf).transpose(0, 2, 1))
    kTh = np.ascontiguousarray(k.astype(bf).transpose(0, 2, 1))
    vTh = np.ascontiguousarray(v.astype(bf).transpose(0, 2, 1))
    Wqb = np.asarray(Wq, dtype=np.float32).astype(bf)
    Wkb = np.asarray(Wk, dtype=np.float32).astype(bf)
    Wvb = np.asarray(Wv, dtype=np.float32).astype(bf)
    Wob = np.asarray(Wo, dtype=np.float32).astype(bf)

    in_maps = []
    for c in range(NC):
        in_maps.append({
            "qT": qTh, "kT": kTh, "vT": vTh,
            "wq": np.ascontiguousarray(Wqb[:, c * DQ:(c + 1) * DQ]),
            "wk": np.ascontiguousarray(Wkb[:, c * DH:(c + 1) * DH]),
            "wv": np.ascontiguousarray(Wvb[:, c * DH:(c + 1) * DH]),
            "wo": np.ascontiguousarray(Wob[c * DQ:(c + 1) * DQ, :]),
        })
    partials = _run(in_maps)
    out = partials.astype(np.float32, copy=False).sum(axis=0)
    return out.reshape(B, S, DM)



# revision 9
# speedup vs baseline: 1.2218x; 1.1179x over previous
"""GQA multi-head attention (B=2, S=2048, D=2048, 32 q-heads / 8 kv-heads)
on 8 Trainium2 NeuronCores.

Sharding: tensor-parallel over kv-head groups. Core c owns kv head c and its
4 query heads: Wq column-shard [2048, 256], Wk/Wv column-shard [2048, 64],
Wo row-shard [256, 2048]. Each core computes a full-shape partial output
(its heads' contribution through Wo); the host sums the 8 partials.

Per-core dataflow (all matmuls bf16 operands, fp32 PSUM accumulate). PE cost
on TRN2 is charged per output-free-size row, so every matmul is oriented to
keep the output free dim minimal for the math it does:
  QT  [256, B*S] = Wq_c^T @ q^T     (q^T fed from host; psum [128, 512])
  K   [keys, 64] = kT-tile^T @ Wk_c (input tile stationary: psum [128keys,64],
                                     64-row charge; K^T recovered via DMA XBAR
                                     transpose into ktd, dup'd to both halves)
  V   [keys, 64] = vT-tile^T @ Wv_c (same flip; lands directly in vsb layout)
  S^T [k, q] = KT-tile.T @ QT       (scores transposed: softmax axis on
                                     partitions)
  expS^T = exp(S^T * 1/8)           (ACT, scale folded in; no max-sub:
                                     |scores/8| small so fp32 exp is safe)
  ctx [q, 65] = expS^T-tile.T @ V_aug  (exp tile stationary: 65-row charge;
                                     ones column gives softmax denominator in
                                     free col 64 -> normalization is a fused
                                     per-partition DVE divide in the required
                                     psum->sbuf copy)
  ctx^T via DMA XBAR transpose into ctxT [dq, q]
  out_partial [B*S, 2048] = ctxT-tiles.T @ Wo_c  (psum copies split DVE/Pool)

Schedule: ctx lags scores by one head-step so PE's in-order stream never
waits on ACT exp; batch-1 projections and out-proj groups are interleaved
into batch-0's exp-bound window.
"""
from collections import deque

from contextlib import ExitStack

import numpy as np
import ml_dtypes

import jax

try:
    jax.config.update("jax_compilation_cache_dir", "/tmp/jax_bass_cache")
    jax.config.update("jax_persistent_cache_min_compile_time_secs", 1.0)
except Exception:
    pass

from jax.sharding import Mesh, PartitionSpec, NamedSharding
from jax.experimental.shard_map import shard_map

import concourse.bass as bass
import concourse.mybir as mybir
import concourse.tile as tile
from concourse import bacc, bass2jax

BF16 = mybir.dt.bfloat16
F32 = mybir.dt.float32
AF = mybir.ActivationFunctionType

B, S, DM = 2, 2048, 2048
HKV, G, DH = 8, 4, 64
DQ = G * DH            # 256: per-core q-projection width
NC = 8
DT = DM // 128         # 16 contraction tiles
BS = B * S             # 4096
SCALE = 1.0 / 8.0      # 1/sqrt(64)

_cache = {}

import os as _os
# "pe" | "dma": the DMA XBAR route is numerically correct in CoreSim but
# races on the compiled NEFF path, so PE transposes are the default.
TRANS = _os.environ.get("K_TRANS", "pe")


def _emit(ctx, tc, qT, kT, vT, wq, wk, wv, wo, out):
    nc = tc.nc

    pp = ctx.enter_context(tc.tile_pool(name="persist", bufs=1))
    wq_sb = pp.tile([128, DT, DQ], BF16, tag="wq")
    wk_sb = pp.tile([128, DT, DH], BF16, tag="wk")
    wv_sb = pp.tile([128, DT, DH], BF16, tag="wv")
    wo_sb = pp.tile([128, 2, DM], BF16, tag="wo")
    qtp = pp.tile([128, 2, BS], BF16, tag="qtp")    # QT pairs [p, hp, b*S+s]
    ktd = pp.tile([128, BS], BF16, tag="ktd")       # KT duplicated both halves
    vsb = pp.tile([128, BS // 128, DH + 1], BF16, tag="vsb")  # V + ones col
    ctxT = pp.tile([128, 2, BS], BF16, tag="ctxT")  # normalized ctx^T pairs

    if TRANS == "pe":
        ident = pp.tile([128, 128], BF16, tag="ident")
        from concourse.masks import make_identity
        make_identity(nc, ident[:])

    # Weight DMA issue order matters: DMA sem-waits hold the SP sequencer,
    # and the lead-in is gated by K/V arrival. wk/wv/wq first (tiny), K/V
    # chunks next, wo deferred past the first q chunk.
    nc.sync.dma_start(wk_sb[:], wk.rearrange("(dt p) m -> p dt m", p=128))
    nc.sync.dma_start(wv_sb[:], wv.rearrange("(dt p) m -> p dt m", p=128))
    nc.sync.dma_start(wq_sb[:], wq.rearrange("(dt p) m -> p dt m", p=128))
    nc.gpsimd.memset(vsb[:, :, DH], 1.0)

    stage = ctx.enter_context(tc.tile_pool(name="stage", bufs=2))
    expp = ctx.enter_context(tc.tile_pool(name="expp", bufs=2))
    smal = ctx.enter_context(tc.tile_pool(name="small", bufs=2))
    outp = ctx.enter_context(tc.tile_pool(name="outp", bufs=2))
    psum = ctx.enter_context(tc.tile_pool(name="psum", bufs=1, space="PSUM"))

    DIV = mybir.AluOpType.divide

    # ---- work-piece pump: ~1-2.5us PE pieces round-robined between the
    # scores pieces of each slot so the 4-deep ACT wait queue never starves.
    work = deque()

    def pump_one():
        while work:
            try:
                next(work[0])
                return True
            except StopIteration:
                work.popleft()
        return False

    def run_gen(g):
        for _ in g:
            pass

    def advance(g):
        return lambda: next(g, None)

    # ---- Phase A emitters -------------------------------------------------
    def a_kv(b, qc):
        """K/V projections, input-tile-stationary: psum [128 keys, 64].
        DMA issue is eager (at call time); compute comes as gen pieces."""
        bo, so = b * S, qc * 512
        k_ch = stage.tile([128, DT, 512], BF16, tag="kv_st", bufs=3,
                          name=f"kch_{b}_{qc}")
        nc.sync.dma_start(
            k_ch[:],
            kT[b].rearrange("(dt p) s -> p dt s", p=128)[:, :, so:so + 512])
        v_ch = stage.tile([128, DT, 512], BF16, tag="kv_st", bufs=3,
                          name=f"vch_{b}_{qc}")
        nc.sync.dma_start(
            v_ch[:],
            vT[b].rearrange("(dt p) s -> p dt s", p=128)[:, :, so:so + 512])

        def gen():
            kv = psum.tile([128, 4, 2, DH], F32, tag="pa", bufs=2,
                           name=f"kv_{b}_{qc}")
            for ki in range(4):
                for dt in range(DT):
                    nc.tensor.matmul(
                        kv[:, ki, 0, :], k_ch[:, dt, ki * 128:(ki + 1) * 128],
                        wk_sb[:, dt, :], start=(dt == 0), stop=(dt == DT - 1))
                for dt in range(DT):
                    nc.tensor.matmul(
                        kv[:, ki, 1, :], v_ch[:, dt, ki * 128:(ki + 1) * 128],
                        wv_sb[:, dt, :], start=(dt == 0), stop=(dt == DT - 1))
                yield
            nc.vector.tensor_copy(
                vsb[:, b * 16 + qc * 4:b * 16 + qc * 4 + 4, 0:DH],
                kv[:, :, 1, :])
            ktmp = smal.tile([128, 4, DH], BF16, tag="ktmp", bufs=2,
                             name=f"ktmp_{b}_{qc}")
            nc.vector.tensor_copy(ktmp[:], kv[:, :, 0, :])
            yield
            if TRANS == "dma":
                for pr in range(2):
                    src = ktmp[:, 2 * pr:2 * pr + 2, :]
                    for half in range(2):
                        dst = ktd[DH * half:DH * half + DH,
                                  bo + so + pr * 256:bo + so + (pr + 1) * 256]
                        nc.sync.dma_start(
                            dst.rearrange("p (t k) -> p t k", t=2), src,
                            transpose=True)
            else:
                for ki in range(4):
                    koff = bo + so + ki * 128
                    ktp = psum.tile([128, 128], BF16, tag="pc", bufs=2,
                                    name=f"ktp_{b}_{qc}_{ki}")
                    for half in range(2):
                        nc.tensor.transpose(
                            ktp[DH * half:DH * half + DH, :], ktmp[:, ki, :],
                            ident[:], tile_position=(0, DH * half))
                    nc.vector.tensor_copy(ktd[:, koff:koff + 128], ktp[:])
            yield

        return gen()

    def a_q_dma(b, qc):
        bo, so = b * S, qc * 512
        q_ch = stage.tile([128, DT, 512], BF16, tag="q_st", bufs=2,
                          name=f"qch_{b}_{qc}")
        nc.sync.dma_start(
            q_ch[:],
            qT[b].rearrange("(dt p) s -> p dt s", p=128)[:, :, so:so + 512])
        return q_ch

    def a_q_proj(b, qc, q_ch):
        bo, so = b * S, qc * 512
        for m in range(2):
            pq = psum.tile([128, 512], F32, tag="pa", bufs=2,
                           name=f"pq_{b}_{qc}_{m}")
            for dt in range(DT):
                nc.tensor.matmul(
                    pq[:], wq_sb[:, dt, m * 128:(m + 1) * 128],
                    q_ch[:, dt, :], start=(dt == 0), stop=(dt == DT - 1))
            nc.vector.tensor_copy(qtp[:, m, bo + so:bo + so + 512], pq[:])
            yield

    # ---- Phase C piece ----------------------------------------------------
    def c_st(b, qc, qt):
        st = b * 16 + qc * 4 + qt
        ost = outp.tile([128, DM], BF16, tag="ost", bufs=3, name=f"ost_{st}")
        for ch in range(4):
            po = psum.tile([128, 512], F32, tag="pa", bufs=2,
                           name=f"po_{st}_{ch}")
            for i in range(2):
                nc.tensor.matmul(
                    po[:], ctxT[:, i, st * 128:(st + 1) * 128],
                    wo_sb[:, i, ch * 512:(ch + 1) * 512],
                    start=(i == 0), stop=(i == 1))
            nc.vector.tensor_copy(ost[:, ch * 512:(ch + 1) * 512], po[:])
        nc.sync.dma_start(out[st * 128:(st + 1) * 128, :], ost[:])

    # ---- Phase B emitters -------------------------------------------------
    cn_map = {}

    def ctx_gen(b, qc, h, ex):
        """ctx [q, 65] with exp tile stationary; normalization fused into the
        required psum->sbuf copy; per-qt transposes + out-proj pieces."""
        i, j = h // 2, h % 2
        if j == 0:
            cn_map[(b, qc, i)] = [
                smal.tile([128, 2, DH], BF16, tag="cn", bufs=10,
                          name=f"cn_{b}_{qc}_{i}_{qt}") for qt in range(4)]
        cn = cn_map[(b, qc, i)]
        pcx = psum.tile([128, 4, DH + 1], F32, tag="pc", bufs=2,
                        name=f"pcx_{b}_{qc}_{h}")
        for qt2 in range(2):
            for qt in (2 * qt2, 2 * qt2 + 1):
                for kt in range(DT):
                    nc.tensor.matmul(
                        pcx[:, qt, :], ex[:, kt, qt * 128:(qt + 1) * 128],
                        vsb[:, b * 16 + kt, :],
                        start=(kt == 0), stop=(kt == DT - 1))
            yield
        rr = smal.tile([128, 4], F32, tag="rr", bufs=3, name=f"rr_{b}_{qc}_{h}")
        nc.vector.reciprocal(rr[:], pcx[:, :, DH])
        for qt in range(4):
            nc.vector.tensor_scalar_mul(
                cn[qt][:, j, :], pcx[:, qt, 0:DH], rr[:, qt:qt + 1])
        yield
        if j == 1:
            qoff = b * S + qc * 512
            for qt in range(4):
                dst = ctxT[:, i, qoff + qt * 128:qoff + (qt + 1) * 128]
                if TRANS == "dma":
                    nc.sync.dma_start(dst, cn[qt][:], transpose=True)
                else:
                    ctp = psum.tile([128, 128], BF16, tag="pc", bufs=2,
                                    name=f"ctp_{b}_{qc}_{i}_{qt}")
                    nc.tensor.transpose(ctp[:], cn[qt][:], ident[:])
                    nc.vector.tensor_copy(dst, ctp[:])
                if i == 1:
                    c_st(b, qc, qt)
                yield
            del cn_map[(b, qc, i)]

    def scores_slot(b, qc, h, actions):
        """Emit the 8 score/exp pieces of slot (qc, h), interleaving one
        action (forced work or deque pump) after each piece."""
        m, j = h // 2, h % 2
        bo = b * S
        qoff = bo + qc * 512
        ex = expp.tile([128, DT, 512], BF16, tag="exp", bufs=2,
                       name=f"ex_{b}_{qc}_{h}")
        for kt2 in range(DT // 2):
            pss = psum.tile([128, 2, 512], F32, tag="sc", bufs=2,
                            name=f"pss_{b}_{qc}_{h}_{kt2}")
            for t in range(2):
                koff = bo + (2 * kt2 + t) * 128
                nc.tensor.matmul(
                    pss[:, t, :], ktd[j * DH:(j + 1) * DH, koff:koff + 128],
                    qtp[j * DH:(j + 1) * DH, m, qoff:qoff + 512])
            nc.scalar.activation(
                ex[:, 2 * kt2:2 * kt2 + 2, :], pss[:], AF.Exp, scale=SCALE)
            if kt2 < len(actions):
                actions[kt2]()
            else:
                pump_one()
        for act in actions[DT // 2:]:
            act()
        return ex

    # ---- Schedule ---------------------------------------------------------
    # Lead-in: K/V chunks stream while projections chase; first q chunk's
    # DMA is slotted before the last K/V pair so q-proj overlaps the tail.
    run_gen(a_kv(0, 0))
    run_gen(a_kv(0, 1))
    run_gen(a_kv(0, 2))
    g_last = a_kv(0, 3)            # issues k3/v3 DMA
    q_ch0 = a_q_dma(0, 0)
    run_gen(g_last)
    run_gen(a_q_proj(0, 0, q_ch0))
    nc.sync.dma_start(wo_sb[:], wo.rearrange("(i p) d -> p i d", p=128))

    q_chunks = {}
    q_projs = {}

    for b in range(B):
        pend = deque()
        for qc in range(4):
            for h in range(4):
                actions = []
                if h == 0 and qc < 3:
                    # q chunk (b, qc+1): DMA now, proj pieces forced next slot
                    q_chunks[(b, qc + 1)] = a_q_dma(b, qc + 1)
                    g = a_q_proj(b, qc + 1, q_chunks[(b, qc + 1)])
                    q_projs[(b, qc + 1)] = g
                    actions += [advance(g), advance(g)]
                if b == 0 and h == 1:
                    g = a_kv(1, qc)
                    actions += [advance(g)] * 6
                if b == 0 and qc == 3 and h == 2:
                    q_chunks[(1, 0)] = a_q_dma(1, 0)
                if pend:
                    work.append(ctx_gen(b, *pend.popleft()))
                ex = scores_slot(b, qc, h, actions)
                pend.append((qc, h, ex))
        work.append(ctx_gen(b, *pend.popleft()))
        if b == 0:
            run_gen(a_q_proj(1, 0, q_chunks[(1, 0)]))
        while pump_one():
            pass


def _build():
    nc = bacc.Bacc("TRN2", target_bir_lowering=False, debug=False, num_devices=NC)
    qT = nc.dram_tensor("qT", [B, DM, S], BF16, kind="ExternalInput")
    kT = nc.dram_tensor("kT", [B, DM, S], BF16, kind="ExternalInput")
    vT = nc.dram_tensor("vT", [B, DM, S], BF16, kind="ExternalInput")
    wq = nc.dram_tensor("wq", [DM, DQ], BF16, kind="ExternalInput")
    wk = nc.dram_tensor("wk", [DM, DH], BF16, kind="ExternalInput")
    wv = nc.dram_tensor("wv", [DM, DH], BF16, kind="ExternalInput")
    wo = nc.dram_tensor("wo", [DQ, DM], BF16, kind="ExternalInput")
    out = nc.dram_tensor("out", [BS, DM], BF16, kind="ExternalOutput")
    with tile.TileContext(nc) as tc:
        with ExitStack() as ctx:
            _emit(ctx, tc, qT.ap(), kT.ap(), vT.ap(), wq.ap(), wk.ap(),
                  wv.ap(), wo.ap(), out.ap())
    nc.compile()
    return nc


def _make_runner(nc, n_cores=NC):
    """Build the sharded jit callable once; reuse across kernel() calls."""
    bass2jax.install_neuronx_cc_hook()
    partition_name = nc.partition_id_tensor.name if nc.partition_id_tensor else None
    in_names, out_names, out_avals, zero_outs = [], [], [], []
    for alloc in nc.m.functions[0].allocations:
        if not isinstance(alloc, mybir.MemoryLocationSet):
            continue
        name = alloc.memorylocations[0].name
        if alloc.kind == "ExternalInput":
            if name != partition_name:
                in_names.append(name)
        elif alloc.kind == "ExternalOutput":
            out_names.append(name)
            shape = tuple(alloc.tensor_shape)
            dtype = mybir.dt.np(alloc.dtype)
            out_avals.append(jax.core.ShapedArray(shape, dtype))
            zero_outs.append(np.zeros(shape, dtype))
    n_params = len(in_names)
    n_outs = len(out_avals)
    in_names_all = in_names + out_names
    if partition_name is not None:
        in_names_all.append(partition_name)
    donate = tuple(range(n_params, n_params + n_outs))

    def _body(*args):
        operands = list(args)
        if partition_name is not None:
            operands.append(bass2jax.partition_id_tensor())
        outs = bass2jax._bass_exec_p.bind(
            *operands,
            out_avals=tuple(out_avals),
            in_names=tuple(in_names_all),
            out_names=tuple(out_names),
            lowering_input_output_aliases=(),
            sim_require_finite=True,
            sim_require_nnan=True,
            nc=nc,
        )
        return tuple(outs)

    devices = jax.devices()[:n_cores]
    mesh = Mesh(np.asarray(devices), ("core",))
    in_specs = (PartitionSpec("core"),) * (n_params + n_outs)
    out_specs = (PartitionSpec("core"),) * len(out_names)
    sharded = jax.jit(
        shard_map(_body, mesh=mesh, in_specs=in_specs, out_specs=out_specs,
                  check_rep=False),
        donate_argnums=donate, keep_unused=True)
    sh = NamedSharding(mesh, PartitionSpec("core"))
    return sharded, in_names, out_names, zero_outs, sh


def _run(in_maps):
    if "nc" not in _cache:
        _cache["nc"] = _build()
    if "runner" not in _cache:
        _cache["runner"] = _make_runner(_cache["nc"])
    sharded, in_names, out_names, zero_outs, sh = _cache["runner"]
    n = NC
    concat_in = [
        jax.device_put(
            np.concatenate([np.asarray(in_maps[c][nm]) for c in range(n)], 0), sh)
        for nm in in_names
    ]
    zeros = [
        jax.device_put(np.zeros((n * z.shape[0], *z.shape[1:]), z.dtype), sh)
        for z in zero_outs
    ]
    outs = sharded(*concat_in, *zeros)
    i = out_names.index("out")
    arr = np.asarray(outs[i])           # [NC*BS, DM]
    return arr.reshape(n, BS, DM)


def kernel(q, k, v, Wq, Wk, Wv, Wo):
    q = np.asarray(q, dtype=np.float32)
    k = np.asarray(k, dtype=np.float32)
    v = np.asarray(v, dtype=np.float32)
    bf = ml_dtypes.bfloat16
    qTh = np.ascontiguousarray(q.astype(bf).transpose(0, 2, 1))
    kTh = np.ascontiguousarray(k.astype(bf).transpose(0, 2, 1))
    vTh = np.ascontiguousarray(v.astype(bf).transpose(0, 2, 1))
    Wqb = np.asarray(Wq, dtype=np.float32).astype(bf)
    Wkb = np.asarray(Wk, dtype=np.float32).astype(bf)
    Wvb = np.asarray(Wv, dtype=np.float32).astype(bf)
    Wob = np.asarray(Wo, dtype=np.float32).astype(bf)

    in_maps = []
    for c in range(NC):
        in_maps.append({
            "qT": qTh, "kT": kTh, "vT": vTh,
            "wq": np.ascontiguousarray(Wqb[:, c * DQ:(c + 1) * DQ]),
            "wk": np.ascontiguousarray(Wkb[:, c * DH:(c + 1) * DH]),
            "wv": np.ascontiguousarray(Wvb[:, c * DH:(c + 1) * DH]),
            "wo": np.ascontiguousarray(Wob[c * DQ:(c + 1) * DQ, :]),
        })
    partials = _run(in_maps)
    out = partials.astype(np.float32, copy=False).sum(axis=0)
    return out.reshape(B, S, DM)



# revision 14
# speedup vs baseline: 1.2804x; 1.0479x over previous
"""GQA multi-head attention (B=2, S=2048, D=2048, 32 q-heads / 8 kv-heads)
on 8 Trainium2 NeuronCores.

Sharding: tensor-parallel over kv-head groups. Core c owns kv head c and its
4 query heads: Wq column-shard [2048, 256], Wk/Wv column-shard [2048, 64],
Wo row-shard [256, 2048]. Each core computes a full-shape partial output
(its heads' contribution through Wo); the host sums the 8 partials.

Per-core dataflow (all matmuls bf16 operands, fp32 PSUM accumulate). PE cost
on TRN2 is charged per output-free-size row, so every matmul is oriented to
keep the output free dim minimal for the math it does:
  QT  [256, B*S] = Wq_c^T @ q^T     (q^T fed from host; psum [128, 512])
  K   [keys, 64] = kT-tile^T @ Wk_c (input tile stationary: psum [128keys,64],
                                     64-row charge; K^T recovered via DMA XBAR
                                     transpose into ktd, dup'd to both halves)
  V   [keys, 64] = vT-tile^T @ Wv_c (same flip; lands directly in vsb layout)
  S^T [k, q] = KT-tile.T @ QT       (scores transposed: softmax axis on
                                     partitions)
  expS^T = exp(S^T * 1/8)           (ACT, scale folded in; no max-sub:
                                     |scores/8| small so fp32 exp is safe)
  ctx [q, 65] = expS^T-tile.T @ V_aug  (exp tile stationary: 65-row charge;
                                     ones column gives softmax denominator in
                                     free col 64 -> normalization is a fused
                                     per-partition DVE divide in the required
                                     psum->sbuf copy)
  ctx^T via DMA XBAR transpose into ctxT [dq, q]
  out_partial [B*S, 2048] = ctxT-tiles.T @ Wo_c  (psum copies split DVE/Pool)

Schedule: ctx lags scores by one head-step so PE's in-order stream never
waits on ACT exp; batch-1 projections and out-proj groups are interleaved
into batch-0's exp-bound window.
"""
from collections import deque

from contextlib import ExitStack

import numpy as np
import ml_dtypes

import jax

try:
    jax.config.update("jax_compilation_cache_dir", "/tmp/jax_bass_cache")
    jax.config.update("jax_persistent_cache_min_compile_time_secs", 1.0)
except Exception:
    pass

from jax.sharding import Mesh, PartitionSpec, NamedSharding
from jax.experimental.shard_map import shard_map

import concourse.bass as bass
import concourse.mybir as mybir
import concourse.tile as tile
from concourse import bacc, bass2jax

BF16 = mybir.dt.bfloat16
F32 = mybir.dt.float32
AF = mybir.ActivationFunctionType

B, S, DM = 2, 2048, 2048
HKV, G, DH = 8, 4, 64
DQ = G * DH            # 256: per-core q-projection width
NC = 8
DT = DM // 128         # 16 contraction tiles
BS = B * S             # 4096
SCALE = 1.0 / 8.0      # 1/sqrt(64)

_cache = {}

import os as _os
# "pe" | "dma": the DMA XBAR route is numerically correct in CoreSim but
# races on the compiled NEFF path, so PE transposes are the default.
TRANS = _os.environ.get("K_TRANS", "pe")


def _emit(ctx, tc, qT, kT, vT, wq, wk, wv, wo, out):
    nc = tc.nc

    pp = ctx.enter_context(tc.tile_pool(name="persist", bufs=1))
    wq_sb = pp.tile([128, DT, DQ], BF16, tag="wq")
    wk_sb = pp.tile([128, DT, DH], BF16, tag="wk")
    wv_sb = pp.tile([128, DT, DH], BF16, tag="wv")
    wo_sb = pp.tile([128, 2, DM], BF16, tag="wo")
    qtp = pp.tile([128, 2, BS], BF16, tag="qtp")    # QT pairs [p, hp, b*S+s]
    ktd = pp.tile([128, BS], BF16, tag="ktd")       # KT duplicated both halves
    vsb = pp.tile([128, BS // 128, DH + 1], BF16, tag="vsb")  # V + ones col
    ctxT = pp.tile([128, 2, BS], BF16, tag="ctxT")  # normalized ctx^T pairs

    if TRANS == "pe":
        ident = pp.tile([128, 128], BF16, tag="ident")
        from concourse.masks import make_identity
        make_identity(nc, ident[:])

    # Weight DMA issue order matters: DMA sem-waits hold the SP sequencer,
    # and the lead-in is gated by K/V arrival. wk/wv/wq first (tiny), K/V
    # chunks next, wo deferred past the first q chunk.
    nc.sync.dma_start(wk_sb[:], wk.rearrange("(dt p) m -> p dt m", p=128))
    nc.sync.dma_start(wv_sb[:], wv.rearrange("(dt p) m -> p dt m", p=128))
    nc.sync.dma_start(wq_sb[:], wq.rearrange("(dt p) m -> p dt m", p=128))
    nc.gpsimd.memset(vsb[:, :, DH], 1.0)

    stage = ctx.enter_context(tc.tile_pool(name="stage", bufs=2))
    expp = ctx.enter_context(tc.tile_pool(name="expp", bufs=2))
    smal = ctx.enter_context(tc.tile_pool(name="small", bufs=2))
    outp = ctx.enter_context(tc.tile_pool(name="outp", bufs=2))
    psum = ctx.enter_context(tc.tile_pool(name="psum", bufs=1, space="PSUM"))

    DIV = mybir.AluOpType.divide

    # ---- work-piece pump: ~1-2.5us PE pieces round-robined between the
    # scores pieces of each slot so the 4-deep ACT wait queue never starves.
    work = deque()

    def pump_one():
        while work:
            try:
                next(work[0])
                return True
            except StopIteration:
                work.popleft()
        return False

    def run_gen(g):
        for _ in g:
            pass

    def advance(g):
        return lambda: next(g, None)

    # ---- Phase A emitters -------------------------------------------------
    def a_kv(b, qc):
        """K/V projections, input-tile-stationary: psum [128 keys, 64].
        DMA issue is eager (at call time); compute comes as gen pieces."""
        bo, so = b * S, qc * 512
        k_ch = stage.tile([128, DT, 512], BF16, tag="kv_st", bufs=3,
                          name=f"kch_{b}_{qc}")
        nc.sync.dma_start(
            k_ch[:],
            kT[b].rearrange("(dt p) s -> p dt s", p=128)[:, :, so:so + 512])
        v_ch = stage.tile([128, DT, 512], BF16, tag="kv_st", bufs=3,
                          name=f"vch_{b}_{qc}")
        nc.sync.dma_start(
            v_ch[:],
            vT[b].rearrange("(dt p) s -> p dt s", p=128)[:, :, so:so + 512])

        def gen():
            kv = psum.tile([128, 4, 2, DH], F32, tag="pa", bufs=2,
                           name=f"kv_{b}_{qc}")
            for ki in range(4):
                for dt in range(DT):
                    nc.tensor.matmul(
                        kv[:, ki, 0, :], k_ch[:, dt, ki * 128:(ki + 1) * 128],
                        wk_sb[:, dt, :], start=(dt == 0), stop=(dt == DT - 1))
                for dt in range(DT):
                    nc.tensor.matmul(
                        kv[:, ki, 1, :], v_ch[:, dt, ki * 128:(ki + 1) * 128],
                        wv_sb[:, dt, :], start=(dt == 0), stop=(dt == DT - 1))
                yield
            nc.vector.tensor_copy(
                vsb[:, b * 16 + qc * 4:b * 16 + qc * 4 + 4, 0:DH],
                kv[:, :, 1, :])
            ktmp = smal.tile([128, 4, DH], BF16, tag="ktmp", bufs=2,
                             name=f"ktmp_{b}_{qc}")
            nc.vector.tensor_copy(ktmp[:], kv[:, :, 0, :])
            yield
            if TRANS == "dma":
                for pr in range(2):
                    src = ktmp[:, 2 * pr:2 * pr + 2, :]
                    for half in range(2):
                        dst = ktd[DH * half:DH * half + DH,
                                  bo + so + pr * 256:bo + so + (pr + 1) * 256]
                        nc.sync.dma_start(
                            dst.rearrange("p (t k) -> p t k", t=2), src,
                            transpose=True)
            else:
                for ki in range(4):
                    koff = bo + so + ki * 128
                    ktp = psum.tile([128, 128], BF16, tag="pc", bufs=2,
                                    name=f"ktp_{b}_{qc}_{ki}")
                    for half in range(2):
                        nc.tensor.transpose(
                            ktp[DH * half:DH * half + DH, :], ktmp[:, ki, :],
                            ident[:], tile_position=(0, DH * half))
                    nc.vector.tensor_copy(ktd[:, koff:koff + 128], ktp[:])
            yield

        return gen()

    def a_q_dma(b, qc):
        bo, so = b * S, qc * 512
        q_ch = stage.tile([128, DT, 512], BF16, tag="q_st", bufs=2,
                          name=f"qch_{b}_{qc}")
        nc.sync.dma_start(
            q_ch[:],
            qT[b].rearrange("(dt p) s -> p dt s", p=128)[:, :, so:so + 512])
        return q_ch

    def a_q_proj(b, qc, q_ch):
        bo, so = b * S, qc * 512
        for m in range(2):
            pq = psum.tile([128, 512], F32, tag="pa", bufs=2,
                           name=f"pq_{b}_{qc}_{m}")
            for dt in range(DT):
                nc.tensor.matmul(
                    pq[:], wq_sb[:, dt, m * 128:(m + 1) * 128],
                    q_ch[:, dt, :], start=(dt == 0), stop=(dt == DT - 1))
            nc.vector.tensor_copy(qtp[:, m, bo + so:bo + so + 512], pq[:])
            yield

    # ---- Phase C piece ----------------------------------------------------
    def c_st(b, qc, qt):
        st = b * 16 + qc * 4 + qt
        ost = outp.tile([128, DM], BF16, tag="ost", bufs=3, name=f"ost_{st}")
        for ch in range(4):
            po = psum.tile([128, 512], F32, tag="pa", bufs=2,
                           name=f"po_{st}_{ch}")
            for i in range(2):
                nc.tensor.matmul(
                    po[:], ctxT[:, i, st * 128:(st + 1) * 128],
                    wo_sb[:, i, ch * 512:(ch + 1) * 512],
                    start=(i == 0), stop=(i == 1))
            nc.vector.tensor_copy(ost[:, ch * 512:(ch + 1) * 512], po[:])
        nc.sync.dma_start(out[st * 128:(st + 1) * 128, :], ost[:])

    # ---- Phase B emitters -------------------------------------------------
    cn_map = {}
    c_backlog = []

    def ctx_gen(b, qc, h, ex, defer_c=False):
        """ctx [q, 65] with exp tile stationary; normalization fused into the
        required psum->sbuf copy; per-qt transposes + out-proj pieces."""
        i, j = h // 2, h % 2
        if j == 0:
            cn_map[(b, qc, i)] = [
                smal.tile([128, 2, DH], BF16, tag="cn", bufs=10,
                          name=f"cn_{b}_{qc}_{i}_{qt}") for qt in range(4)]
        cn = cn_map[(b, qc, i)]
        pcx = psum.tile([128, 4, DH + 1], F32, tag="pc", bufs=2,
                        name=f"pcx_{b}_{qc}_{h}")
        for qt2 in range(2):
            for qt in (2 * qt2, 2 * qt2 + 1):
                for kt in range(DT):
                    nc.tensor.matmul(
                        pcx[:, qt, :], ex[:, kt, qt * 128:(qt + 1) * 128],
                        vsb[:, b * 16 + kt, :],
                        start=(kt == 0), stop=(kt == DT - 1))
            yield
        rr = smal.tile([128, 4], F32, tag="rr", bufs=3, name=f"rr_{b}_{qc}_{h}")
        nc.vector.reciprocal(rr[:], pcx[:, :, DH])
        for qt in range(4):
            nc.vector.tensor_scalar_mul(
                cn[qt][:, j, :], pcx[:, qt, 0:DH], rr[:, qt:qt + 1])
        yield
        if j == 1:
            qoff = b * S + qc * 512
            for qt in range(4):
                dst = ctxT[:, i, qoff + qt * 128:qoff + (qt + 1) * 128]
                if TRANS == "dma":
                    nc.sync.dma_start(dst, cn[qt][:], transpose=True)
                else:
                    ctp = psum.tile([128, 128], BF16, tag="pc", bufs=2,
                                    name=f"ctp_{b}_{qc}_{i}_{qt}")
                    nc.tensor.transpose(ctp[:], cn[qt][:], ident[:])
                    nc.vector.tensor_copy(dst, ctp[:])
                if i == 1:
                    if defer_c:
                        c_backlog.append((b, qc, qt))
                    else:
                        c_st(b, qc, qt)
                yield
            del cn_map[(b, qc, i)]

    def scores_slot(b, qc, h, actions):
        """Emit the 8 score/exp pieces of slot (qc, h), interleaving one
        action (forced work or deque pump) after each piece."""
        m, j = h // 2, h % 2
        bo = b * S
        qoff = bo + qc * 512
        ex = expp.tile([128, DT, 512], BF16, tag="exp", bufs=2,
                       name=f"ex_{b}_{qc}_{h}")
        for kt2 in range(DT // 2):
            pss = psum.tile([128, 2, 512], F32, tag="sc", bufs=2,
                            name=f"pss_{b}_{qc}_{h}_{kt2}")
            for t in range(2):
                koff = bo + (2 * kt2 + t) * 128
                nc.tensor.matmul(
                    pss[:, t, :], ktd[j * DH:(j + 1) * DH, koff:koff + 128],
                    qtp[j * DH:(j + 1) * DH, m, qoff:qoff + 512])
            nc.scalar.activation(
                ex[:, 2 * kt2:2 * kt2 + 2, :], pss[:], AF.Exp, scale=SCALE)
            # Skip the first two pieces: the previous slot's exp tail is
            # still in flight then, and a pumped ctx piece would stall PE.
            if kt2 >= 2:
                k = kt2 - 2
                if k < len(actions):
                    actions[k]()
                else:
                    pump_one()
        for act in actions[DT // 2 - 2:]:
            act()
        pump_one()
        pump_one()
        return ex

    # ---- Schedule ---------------------------------------------------------
    # Lead-in: K/V chunks stream while projections chase; first q chunk's
    # DMA is slotted before the last K/V pair so q-proj overlaps the tail.
    run_gen(a_kv(0, 0))
    run_gen(a_kv(0, 1))
    run_gen(a_kv(0, 2))
    g_last = a_kv(0, 3)            # issues k3/v3 DMA
    q_ch0 = a_q_dma(0, 0)
    run_gen(g_last)
    run_gen(a_q_proj(0, 0, q_ch0))
    nc.sync.dma_start(wo_sb[:], wo.rearrange("(i p) d -> p i d", p=128))

    q_chunks = {}
    q_projs = {}

    for b in range(B):
        pend = deque()
        for qc in range(4):
            for h in range(4):
                actions = []
                if h == 0 and qc < 3:
                    # q chunk (b, qc+1): DMA now, proj pieces forced next slot
                    q_chunks[(b, qc + 1)] = a_q_dma(b, qc + 1)
                    g = a_q_proj(b, qc + 1, q_chunks[(b, qc + 1)])
                    q_projs[(b, qc + 1)] = g
                    actions += [advance(g), advance(g)]
                if b == 0 and h == 1:
                    g = a_kv(1, qc)
                    actions += [advance(g)] * 6
                if b == 0 and qc == 3 and h == 2:
                    q_chunks[(1, 0)] = a_q_dma(1, 0)
                if b == 1 and c_backlog:
                    bb, bqc, bqt = c_backlog.pop(0)
                    actions.append(
                        lambda bb=bb, bqc=bqc, bqt=bqt: c_st(bb, bqc, bqt))
                if pend:
                    prev = pend.popleft()
                    work.append(ctx_gen(b, *prev,
                                        defer_c=(b == 0 and prev[0] >= 2)))
                ex = scores_slot(b, qc, h, actions)
                pend.append((qc, h, ex))
        prev = pend.popleft()
        work.append(ctx_gen(b, *prev, defer_c=(b == 0)))
        if b == 0:
            run_gen(a_q_proj(1, 0, q_chunks[(1, 0)]))
        while pump_one():
            pass


def _build():
    nc = bacc.Bacc("TRN2", target_bir_lowering=False, debug=False, num_devices=NC)
    qT = nc.dram_tensor("qT", [B, DM, S], BF16, kind="ExternalInput")
    kT = nc.dram_tensor("kT", [B, DM, S], BF16, kind="ExternalInput")
    vT = nc.dram_tensor("vT", [B, DM, S], BF16, kind="ExternalInput")
    wq = nc.dram_tensor("wq", [DM, DQ], BF16, kind="ExternalInput")
    wk = nc.dram_tensor("wk", [DM, DH], BF16, kind="ExternalInput")
    wv = nc.dram_tensor("wv", [DM, DH], BF16, kind="ExternalInput")
    wo = nc.dram_tensor("wo", [DQ, DM], BF16, kind="ExternalInput")
    out = nc.dram_tensor("out", [BS, DM], BF16, kind="ExternalOutput")
    with tile.TileContext(nc) as tc:
        with ExitStack() as ctx:
            _emit(ctx, tc, qT.ap(), kT.ap(), vT.ap(), wq.ap(), wk.ap(),
                  wv.ap(), wo.ap(), out.ap())
    nc.compile()
    return nc


def _make_runner(nc, n_cores=NC):
    """Build the sharded jit callable once; reuse across kernel() calls."""
    bass2jax.install_neuronx_cc_hook()
    partition_name = nc.partition_id_tensor.name if nc.partition_id_tensor else None
    in_names, out_names, out_avals, zero_outs = [], [], [], []
    for alloc in nc.m.functions[0].allocations:
        if not isinstance(alloc, mybir.MemoryLocationSet):
            continue
        name = alloc.memorylocations[0].name
        if alloc.kind == "ExternalInput":
            if name != partition_name:
                in_names.append(name)
        elif alloc.kind == "ExternalOutput":
            out_names.append(name)
            shape = tuple(alloc.tensor_shape)
            dtype = mybir.dt.np(alloc.dtype)
            out_avals.append(jax.core.ShapedArray(shape, dtype))
            zero_outs.append(np.zeros(shape, dtype))
    n_params = len(in_names)
    n_outs = len(out_avals)
    in_names_all = in_names + out_names
    if partition_name is not None:
        in_names_all.append(partition_name)
    donate = tuple(range(n_params, n_params + n_outs))

    def _body(*args):
        operands = list(args)
        if partition_name is not None:
            operands.append(bass2jax.partition_id_tensor())
        outs = bass2jax._bass_exec_p.bind(
            *operands,
            out_avals=tuple(out_avals),
            in_names=tuple(in_names_all),
            out_names=tuple(out_names),
            lowering_input_output_aliases=(),
            sim_require_finite=True,
            sim_require_nnan=True,
            nc=nc,
        )
        return tuple(outs)

    devices = jax.devices()[:n_cores]
    mesh = Mesh(np.asarray(devices), ("core",))
    in_specs = (PartitionSpec("core"),) * (n_params + n_outs)
    out_specs = (PartitionSpec("core"),) * len(out_names)
    sharded = jax.jit(
        shard_map(_body, mesh=mesh, in_specs=in_specs, out_specs=out_specs,
                  check_rep=False),
        donate_argnums=donate, keep_unused=True)
    sh = NamedSharding(mesh, PartitionSpec("core"))
    return sharded, in_names, out_names, zero_outs, sh


def _run(in_maps):
    if "nc" not in _cache:
        _cache["nc"] = _build()
    if "runner" not in _cache:
        _cache["runner"] = _make_runner(_cache["nc"])
    sharded, in_names, out_names, zero_outs, sh = _cache["runner"]
    n = NC
    concat_in = [
        jax.device_put(
            np.concatenate([np.asarray(in_maps[c][nm]) for c in range(n)], 0), sh)
        for nm in in_names
    ]
    zeros = [
        jax.device_put(np.zeros((n * z.shape[0], *z.shape[1:]), z.dtype), sh)
        for z in zero_outs
    ]
    outs = sharded(*concat_in, *zeros)
    i = out_names.index("out")
    arr = np.asarray(outs[i])           # [NC*BS, DM]
    return arr.reshape(n, BS, DM)


def kernel(q, k, v, Wq, Wk, Wv, Wo):
    q = np.asarray(q, dtype=np.float32)
    k = np.asarray(k, dtype=np.float32)
    v = np.asarray(v, dtype=np.float32)
    bf = ml_dtypes.bfloat16
    qTh = np.ascontiguousarray(q.astype(bf).transpose(0, 2, 1))
    kTh = np.ascontiguousarray(k.astype(bf).transpose(0, 2, 1))
    vTh = np.ascontiguousarray(v.astype(bf).transpose(0, 2, 1))
    Wqb = np.asarray(Wq, dtype=np.float32).astype(bf)
    Wkb = np.asarray(Wk, dtype=np.float32).astype(bf)
    Wvb = np.asarray(Wv, dtype=np.float32).astype(bf)
    Wob = np.asarray(Wo, dtype=np.float32).astype(bf)

    in_maps = []
    for c in range(NC):
        in_maps.append({
            "qT": qTh, "kT": kTh, "vT": vTh,
            "wq": np.ascontiguousarray(Wqb[:, c * DQ:(c + 1) * DQ]),
            "wk": np.ascontiguousarray(Wkb[:, c * DH:(c + 1) * DH]),
            "wv": np.ascontiguousarray(Wvb[:, c * DH:(c + 1) * DH]),
            "wo": np.ascontiguousarray(Wob[c * DQ:(c + 1) * DQ, :]),
        })
    partials = _run(in_maps)
    out = partials.astype(np.float32, copy=False).sum(axis=0)
    return out.reshape(B, S, DM)



# revision 15
# speedup vs baseline: 1.2826x; 1.0017x over previous
"""GQA multi-head attention (B=2, S=2048, D=2048, 32 q-heads / 8 kv-heads)
on 8 Trainium2 NeuronCores.

Sharding: tensor-parallel over kv-head groups. Core c owns kv head c and its
4 query heads: Wq column-shard [2048, 256], Wk/Wv column-shard [2048, 64],
Wo row-shard [256, 2048]. Each core computes a full-shape partial output
(its heads' contribution through Wo); the host sums the 8 partials.

Per-core dataflow (all matmuls bf16 operands, fp32 PSUM accumulate). PE cost
on TRN2 is charged per output-free-size row, so every matmul is oriented to
keep the output free dim minimal for the math it does:
  QT  [256, B*S] = Wq_c^T @ q^T     (q^T fed from host; psum [128, 512])
  K   [keys, 64] = kT-tile^T @ Wk_c (input tile stationary: psum [128keys,64],
                                     64-row charge; K^T recovered via DMA XBAR
                                     transpose into ktd, dup'd to both halves)
  V   [keys, 64] = vT-tile^T @ Wv_c (same flip; lands directly in vsb layout)
  S^T [k, q] = KT-tile.T @ QT       (scores transposed: softmax axis on
                                     partitions)
  expS^T = exp(S^T * 1/8)           (ACT, scale folded in; no max-sub:
                                     |scores/8| small so fp32 exp is safe)
  ctx [q, 65] = expS^T-tile.T @ V_aug  (exp tile stationary: 65-row charge;
                                     ones column gives softmax denominator in
                                     free col 64 -> normalization is a fused
                                     per-partition DVE divide in the required
                                     psum->sbuf copy)
  ctx^T via DMA XBAR transpose into ctxT [dq, q]
  out_partial [B*S, 2048] = ctxT-tiles.T @ Wo_c  (psum copies split DVE/Pool)

Schedule: ctx lags scores by one head-step so PE's in-order stream never
waits on ACT exp; batch-1 projections and out-proj groups are interleaved
into batch-0's exp-bound window.
"""
from collections import deque

from contextlib import ExitStack

import numpy as np
import ml_dtypes

import jax

try:
    jax.config.update("jax_compilation_cache_dir", "/tmp/jax_bass_cache")
    jax.config.update("jax_persistent_cache_min_compile_time_secs", 1.0)
except Exception:
    pass

from jax.sharding import Mesh, PartitionSpec, NamedSharding
from jax.experimental.shard_map import shard_map

import concourse.bass as bass
import concourse.mybir as mybir
import concourse.tile as tile
from concourse import bacc, bass2jax

BF16 = mybir.dt.bfloat16
FP8 = mybir.dt.float8e4
F32 = mybir.dt.float32
AF = mybir.ActivationFunctionType

B, S, DM = 2, 2048, 2048
HKV, G, DH = 8, 4, 64
DQ = G * DH            # 256: per-core q-projection width
NC = 8
DT = DM // 128         # 16 contraction tiles
BS = B * S             # 4096
SCALE = 1.0 / 8.0      # 1/sqrt(64)

_cache = {}

import os as _os
# "pe" | "dma": the DMA XBAR route is numerically correct in CoreSim but
# races on the compiled NEFF path, so PE transposes are the default.
TRANS = _os.environ.get("K_TRANS", "pe")


def _emit(ctx, tc, qTh, qTl, kT, vT, wqh, wql, wk, wv, wo, out):
    nc = tc.nc

    pp = ctx.enter_context(tc.tile_pool(name="persist", bufs=1))
    wqh_sb = pp.tile([128, DT, DQ], FP8, tag="wqh")
    wql_sb = pp.tile([128, DT, DQ], FP8, tag="wql")
    wk_sb = pp.tile([128, DT, DH], BF16, tag="wk")
    wv_sb = pp.tile([128, DT, DH], BF16, tag="wv")
    wo_sb = pp.tile([128, 2, DM], BF16, tag="wo")
    qtp = pp.tile([128, 2, BS], BF16, tag="qtp")    # QT pairs [p, hp, b*S+s]
    ktd = pp.tile([128, BS], BF16, tag="ktd")       # KT duplicated both halves
    vsb = pp.tile([128, BS // 128, DH + 1], BF16, tag="vsb")  # V + ones col
    ctxT = pp.tile([128, 2, BS], BF16, tag="ctxT")  # normalized ctx^T pairs

    if TRANS == "pe":
        ident = pp.tile([128, 128], BF16, tag="ident")
        from concourse.masks import make_identity
        make_identity(nc, ident[:])

    # Weight DMA issue order matters: DMA sem-waits hold the SP sequencer,
    # and the lead-in is gated by K/V arrival. wk/wv/wq first (tiny), K/V
    # chunks next, wo deferred past the first q chunk.
    nc.sync.dma_start(wk_sb[:], wk.rearrange("(dt p) m -> p dt m", p=128))
    nc.sync.dma_start(wv_sb[:], wv.rearrange("(dt p) m -> p dt m", p=128))
    nc.sync.dma_start(wqh_sb[:], wqh.rearrange("(dt p) m -> p dt m", p=128))
    nc.sync.dma_start(wql_sb[:], wql.rearrange("(dt p) m -> p dt m", p=128))
    nc.gpsimd.memset(vsb[:, :, DH], 1.0)

    stage = ctx.enter_context(tc.tile_pool(name="stage", bufs=2))
    expp = ctx.enter_context(tc.tile_pool(name="expp", bufs=2))
    smal = ctx.enter_context(tc.tile_pool(name="small", bufs=2))
    outp = ctx.enter_context(tc.tile_pool(name="outp", bufs=2))
    psum = ctx.enter_context(tc.tile_pool(name="psum", bufs=1, space="PSUM"))

    DIV = mybir.AluOpType.divide

    # ---- work-piece pump: ~1-2.5us PE pieces round-robined between the
    # scores pieces of each slot so the 4-deep ACT wait queue never starves.
    work = deque()

    def pump_one():
        while work:
            try:
                next(work[0])
                return True
            except StopIteration:
                work.popleft()
        return False

    def run_gen(g):
        for _ in g:
            pass

    def advance(g):
        return lambda: next(g, None)

    # ---- Phase A emitters -------------------------------------------------
    def a_kv(b, qc):
        """K/V projections, input-tile-stationary: psum [128 keys, 64].
        DMA issue is eager (at call time); compute comes as gen pieces."""
        bo, so = b * S, qc * 512
        k_ch = stage.tile([128, DT, 512], BF16, tag="kv_st", bufs=3,
                          name=f"kch_{b}_{qc}")
        nc.sync.dma_start(
            k_ch[:],
            kT[b].rearrange("(dt p) s -> p dt s", p=128)[:, :, so:so + 512])
        v_ch = stage.tile([128, DT, 512], BF16, tag="kv_st", bufs=3,
                          name=f"vch_{b}_{qc}")
        nc.sync.dma_start(
            v_ch[:],
            vT[b].rearrange("(dt p) s -> p dt s", p=128)[:, :, so:so + 512])

        def gen():
            kv = psum.tile([128, 4, 2, DH], F32, tag="pa", bufs=2,
                           name=f"kv_{b}_{qc}")
            for ki in range(4):
                for dt in range(DT):
                    nc.tensor.matmul(
                        kv[:, ki, 0, :], k_ch[:, dt, ki * 128:(ki + 1) * 128],
                        wk_sb[:, dt, :], start=(dt == 0), stop=(dt == DT - 1))
                for dt in range(DT):
                    nc.tensor.matmul(
                        kv[:, ki, 1, :], v_ch[:, dt, ki * 128:(ki + 1) * 128],
                        wv_sb[:, dt, :], start=(dt == 0), stop=(dt == DT - 1))
                yield
            nc.vector.tensor_copy(
                vsb[:, b * 16 + qc * 4:b * 16 + qc * 4 + 4, 0:DH],
                kv[:, :, 1, :])
            ktmp = smal.tile([128, 4, DH], BF16, tag="ktmp", bufs=2,
                             name=f"ktmp_{b}_{qc}")
            nc.vector.tensor_copy(ktmp[:], kv[:, :, 0, :])
            yield
            if TRANS == "dma":
                for pr in range(2):
                    src = ktmp[:, 2 * pr:2 * pr + 2, :]
                    for half in range(2):
                        dst = ktd[DH * half:DH * half + DH,
                                  bo + so + pr * 256:bo + so + (pr + 1) * 256]
                        nc.sync.dma_start(
                            dst.rearrange("p (t k) -> p t k", t=2), src,
                            transpose=True)
            else:
                for ki in range(4):
                    koff = bo + so + ki * 128
                    ktp = psum.tile([128, 128], BF16, tag="pc", bufs=2,
                                    name=f"ktp_{b}_{qc}_{ki}")
                    for half in range(2):
                        nc.tensor.transpose(
                            ktp[DH * half:DH * half + DH, :], ktmp[:, ki, :],
                            ident[:], tile_position=(0, DH * half))
                    nc.vector.tensor_copy(ktd[:, koff:koff + 128], ktp[:])
            yield

        return gen()

    def a_q_dma(b, qc):
        bo, so = b * S, qc * 512
        qh_ch = stage.tile([128, DT, 512], FP8, tag="q_st", bufs=4,
                           name=f"qhch_{b}_{qc}")
        nc.sync.dma_start(
            qh_ch[:],
            qTh[b].rearrange("(dt p) s -> p dt s", p=128)[:, :, so:so + 512])
        ql_ch = stage.tile([128, DT, 512], FP8, tag="q_st", bufs=4,
                           name=f"qlch_{b}_{qc}")
        nc.sync.dma_start(
            ql_ch[:],
            qTl[b].rearrange("(dt p) s -> p dt s", p=128)[:, :, so:so + 512])
        return (qh_ch, ql_ch)

    def a_q_proj(b, qc, q_ch):
        bo, so = b * S, qc * 512
        qh_ch, ql_ch = q_ch
        terms = [(wqh_sb, qh_ch), (wqh_sb, ql_ch), (wql_sb, qh_ch)]
        for m in range(2):
            pq = psum.tile([128, 512], F32, tag="pa", bufs=2,
                           name=f"pq_{b}_{qc}_{m}")
            n = 0
            for w_sb, qch in terms:
                for t in range(DT // 2):
                    nc.tensor.matmul(
                        pq[:], w_sb[:, 2 * t:2 * t + 2, m * 128:(m + 1) * 128],
                        qch[:, 2 * t:2 * t + 2, :],
                        start=(n == 0), stop=(n == 3 * DT // 2 - 1),
                        perf_mode=mybir.MatmulPerfMode.DoubleRow)
                    n += 1
            nc.vector.tensor_scalar_mul(
                qtp[:, m, bo + so:bo + so + 512], pq[:], 1.0 / 64.0)
            yield

    # ---- Phase C piece ----------------------------------------------------
    def c_st(b, qc, qt):
        st = b * 16 + qc * 4 + qt
        ost = outp.tile([128, DM], BF16, tag="ost", bufs=3, name=f"ost_{st}")
        for ch in range(4):
            po = psum.tile([128, 512], F32, tag="pa", bufs=2,
                           name=f"po_{st}_{ch}")
            for i in range(2):
                nc.tensor.matmul(
                    po[:], ctxT[:, i, st * 128:(st + 1) * 128],
                    wo_sb[:, i, ch * 512:(ch + 1) * 512],
                    start=(i == 0), stop=(i == 1))
            nc.vector.tensor_copy(ost[:, ch * 512:(ch + 1) * 512], po[:])
        nc.sync.dma_start(out[st * 128:(st + 1) * 128, :], ost[:])

    # ---- Phase B emitters -------------------------------------------------
    cn_map = {}
    c_backlog = []

    def ctx_gen(b, qc, h, ex, defer_c=False):
        """ctx [q, 65] with exp tile stationary; normalization fused into the
        required psum->sbuf copy; per-qt transposes + out-proj pieces."""
        i, j = h // 2, h % 2
        if j == 0:
            cn_map[(b, qc, i)] = [
                smal.tile([128, 2, DH], BF16, tag="cn", bufs=10,
                          name=f"cn_{b}_{qc}_{i}_{qt}") for qt in range(4)]
        cn = cn_map[(b, qc, i)]
        pcx = psum.tile([128, 4, DH + 1], F32, tag="pc", bufs=2,
                        name=f"pcx_{b}_{qc}_{h}")
        for qt2 in range(2):
            for qt in (2 * qt2, 2 * qt2 + 1):
                for kt in range(DT):
                    nc.tensor.matmul(
                        pcx[:, qt, :], ex[:, kt, qt * 128:(qt + 1) * 128],
                        vsb[:, b * 16 + kt, :],
                        start=(kt == 0), stop=(kt == DT - 1))
            yield
        rr = smal.tile([128, 4], F32, tag="rr", bufs=3, name=f"rr_{b}_{qc}_{h}")
        nc.vector.reciprocal(rr[:], pcx[:, :, DH])
        for qt in range(4):
            nc.vector.tensor_scalar_mul(
                cn[qt][:, j, :], pcx[:, qt, 0:DH], rr[:, qt:qt + 1])
        yield
        if j == 1:
            qoff = b * S + qc * 512
            for qt in range(4):
                dst = ctxT[:, i, qoff + qt * 128:qoff + (qt + 1) * 128]
                if TRANS == "dma":
                    nc.sync.dma_start(dst, cn[qt][:], transpose=True)
                else:
                    ctp = psum.tile([128, 128], BF16, tag="pc", bufs=2,
                                    name=f"ctp_{b}_{qc}_{i}_{qt}")
                    nc.tensor.transpose(ctp[:], cn[qt][:], ident[:])
                    nc.vector.tensor_copy(dst, ctp[:])
                if i == 1:
                    if defer_c:
                        c_backlog.append((b, qc, qt))
                    else:
                        c_st(b, qc, qt)
                yield
            del cn_map[(b, qc, i)]

    def scores_slot(b, qc, h, actions):
        """Emit the 8 score/exp pieces of slot (qc, h), interleaving one
        action (forced work or deque pump) after each piece."""
        m, j = h // 2, h % 2
        bo = b * S
        qoff = bo + qc * 512
        ex = expp.tile([128, DT, 512], BF16, tag="exp", bufs=2,
                       name=f"ex_{b}_{qc}_{h}")
        for kt2 in range(DT // 2):
            pss = psum.tile([128, 2, 512], F32, tag="sc", bufs=2,
                            name=f"pss_{b}_{qc}_{h}_{kt2}")
            for t in range(2):
                koff = bo + (2 * kt2 + t) * 128
                nc.tensor.matmul(
                    pss[:, t, :], ktd[j * DH:(j + 1) * DH, koff:koff + 128],
                    qtp[j * DH:(j + 1) * DH, m, qoff:qoff + 512])
            nc.scalar.activation(
                ex[:, 2 * kt2:2 * kt2 + 2, :], pss[:], AF.Exp, scale=SCALE)
            # Skip the first two pieces: the previous slot's exp tail is
            # still in flight then, and a pumped ctx piece would stall PE.
            if kt2 >= 2:
                k = kt2 - 2
                if k < len(actions):
                    actions[k]()
                else:
                    pump_one()
        for act in actions[DT // 2 - 2:]:
            act()
        pump_one()
        pump_one()
        return ex

    # ---- Schedule ---------------------------------------------------------
    # Lead-in: K/V chunks stream while projections chase; first q chunk's
    # DMA is slotted before the last K/V pair so q-proj overlaps the tail.
    run_gen(a_kv(0, 0))
    run_gen(a_kv(0, 1))
    run_gen(a_kv(0, 2))
    g_last = a_kv(0, 3)            # issues k3/v3 DMA
    q_ch0 = a_q_dma(0, 0)
    run_gen(g_last)
    run_gen(a_q_proj(0, 0, q_ch0))
    nc.sync.dma_start(wo_sb[:], wo.rearrange("(i p) d -> p i d", p=128))

    q_chunks = {}
    q_projs = {}

    for b in range(B):
        pend = deque()
        for qc in range(4):
            for h in range(4):
                actions = []
                if h == 0 and qc < 3:
                    # q chunk (b, qc+1): DMA now, proj pieces forced next slot
                    q_chunks[(b, qc + 1)] = a_q_dma(b, qc + 1)
                    g = a_q_proj(b, qc + 1, q_chunks[(b, qc + 1)])
                    q_projs[(b, qc + 1)] = g
                    actions += [advance(g), advance(g)]
                if b == 0 and h == 1:
                    g = a_kv(1, qc)
                    actions += [advance(g)] * 6
                if b == 0 and qc == 3 and h == 2:
                    q_chunks[(1, 0)] = a_q_dma(1, 0)
                if b == 1 and c_backlog:
                    bb, bqc, bqt = c_backlog.pop(0)
                    actions.append(
                        lambda bb=bb, bqc=bqc, bqt=bqt: c_st(bb, bqc, bqt))
                if pend:
                    prev = pend.popleft()
                    work.append(ctx_gen(b, *prev,
                                        defer_c=(b == 0 and prev[0] >= 2)))
                ex = scores_slot(b, qc, h, actions)
                pend.append((qc, h, ex))
        prev = pend.popleft()
        work.append(ctx_gen(b, *prev, defer_c=(b == 0)))
        if b == 0:
            run_gen(a_q_proj(1, 0, q_chunks[(1, 0)]))
        while pump_one():
            pass


def _build():
    nc = bacc.Bacc("TRN2", target_bir_lowering=False, debug=False, num_devices=NC)
    qTh = nc.dram_tensor("qTh", [B, DM, S], FP8, kind="ExternalInput")
    qTl = nc.dram_tensor("qTl", [B, DM, S], FP8, kind="ExternalInput")
    kT = nc.dram_tensor("kT", [B, DM, S], BF16, kind="ExternalInput")
    vT = nc.dram_tensor("vT", [B, DM, S], BF16, kind="ExternalInput")
    wqh = nc.dram_tensor("wqh", [DM, DQ], FP8, kind="ExternalInput")
    wql = nc.dram_tensor("wql", [DM, DQ], FP8, kind="ExternalInput")
    wk = nc.dram_tensor("wk", [DM, DH], BF16, kind="ExternalInput")
    wv = nc.dram_tensor("wv", [DM, DH], BF16, kind="ExternalInput")
    wo = nc.dram_tensor("wo", [DQ, DM], BF16, kind="ExternalInput")
    out = nc.dram_tensor("out", [BS, DM], BF16, kind="ExternalOutput")
    with tile.TileContext(nc) as tc:
        with ExitStack() as ctx:
            _emit(ctx, tc, qTh.ap(), qTl.ap(), kT.ap(), vT.ap(), wqh.ap(),
                  wql.ap(), wk.ap(), wv.ap(), wo.ap(), out.ap())
    nc.compile()
    return nc


def _make_runner(nc, n_cores=NC):
    """Build the sharded jit callable once; reuse across kernel() calls."""
    bass2jax.install_neuronx_cc_hook()
    partition_name = nc.partition_id_tensor.name if nc.partition_id_tensor else None
    in_names, out_names, out_avals, zero_outs = [], [], [], []
    for alloc in nc.m.functions[0].allocations:
        if not isinstance(alloc, mybir.MemoryLocationSet):
            continue
        name = alloc.memorylocations[0].name
        if alloc.kind == "ExternalInput":
            if name != partition_name:
                in_names.append(name)
        elif alloc.kind == "ExternalOutput":
            out_names.append(name)
            shape = tuple(alloc.tensor_shape)
            dtype = mybir.dt.np(alloc.dtype)
            out_avals.append(jax.core.ShapedArray(shape, dtype))
            zero_outs.append(np.zeros(shape, dtype))
    n_params = len(in_names)
    n_outs = len(out_avals)
    in_names_all = in_names + out_names
    if partition_name is not None:
        in_names_all.append(partition_name)
    donate = tuple(range(n_params, n_params + n_outs))

    def _body(*args):
        operands = list(args)
        if partition_name is not None:
            operands.append(bass2jax.partition_id_tensor())
        outs = bass2jax._bass_exec_p.bind(
            *operands,
            out_avals=tuple(out_avals),
            in_names=tuple(in_names_all),
            out_names=tuple(out_names),
            lowering_input_output_aliases=(),
            sim_require_finite=True,
            sim_require_nnan=True,
            nc=nc,
        )
        return tuple(outs)

    devices = jax.devices()[:n_cores]
    mesh = Mesh(np.asarray(devices), ("core",))
    in_specs = (PartitionSpec("core"),) * (n_params + n_outs)
    out_specs = (PartitionSpec("core"),) * len(out_names)
    sharded = jax.jit(
        shard_map(_body, mesh=mesh, in_specs=in_specs, out_specs=out_specs,
                  check_rep=False),
        donate_argnums=donate, keep_unused=True)
    sh = NamedSharding(mesh, PartitionSpec("core"))
    return sharded, in_names, out_names, zero_outs, sh


def _run(in_maps):
    if "nc" not in _cache:
        _cache["nc"] = _build()
    if "runner" not in _cache:
        _cache["runner"] = _make_runner(_cache["nc"])
    sharded, in_names, out_names, zero_outs, sh = _cache["runner"]
    n = NC
    concat_in = [
        jax.device_put(
            np.concatenate([np.asarray(in_maps[c][nm]) for c in range(n)], 0), sh)
        for nm in in_names
    ]
    zeros = [
        jax.device_put(np.zeros((n * z.shape[0], *z.shape[1:]), z.dtype), sh)
        for z in zero_outs
    ]
    outs = sharded(*concat_in, *zeros)
    i = out_names.index("out")
    arr = np.asarray(outs[i])           # [NC*BS, DM]
    return arr.reshape(n, BS, DM)


def kernel(q, k, v, Wq, Wk, Wv, Wo):
    q = np.asarray(q, dtype=np.float32)
    k = np.asarray(k, dtype=np.float32)
    v = np.asarray(v, dtype=np.float32)
    bf = ml_dtypes.bfloat16
    f8 = ml_dtypes.float8_e4m3
    qT32 = np.ascontiguousarray(q.transpose(0, 2, 1))
    qThi = qT32.astype(f8)
    qTlo = (qT32 - qThi.astype(np.float32)).astype(f8)
    kTh = np.ascontiguousarray(k.astype(bf).transpose(0, 2, 1))
    vTh = np.ascontiguousarray(v.astype(bf).transpose(0, 2, 1))
    Wq64 = np.asarray(Wq, dtype=np.float32) * 64.0
    Wqhi = Wq64.astype(f8)
    Wqlo = (Wq64 - Wqhi.astype(np.float32)).astype(f8)
    Wkb = np.asarray(Wk, dtype=np.float32).astype(bf)
    Wvb = np.asarray(Wv, dtype=np.float32).astype(bf)
    Wob = np.asarray(Wo, dtype=np.float32).astype(bf)

    in_maps = []
    for c in range(NC):
        in_maps.append({
            "qTh": qThi, "qTl": qTlo, "kT": kTh, "vT": vTh,
            "wqh": np.ascontiguousarray(Wqhi[:, c * DQ:(c + 1) * DQ]),
            "wql": np.ascontiguousarray(Wqlo[:, c * DQ:(c + 1) * DQ]),
            "wk": np.ascontiguousarray(Wkb[:, c * DH:(c + 1) * DH]),
            "wv": np.ascontiguousarray(Wvb[:, c * DH:(c + 1) * DH]),
            "wo": np.ascontiguousarray(Wob[c * DQ:(c + 1) * DQ, :]),
        })
    partials = _run(in_maps)
    out = partials.astype(np.float32, copy=False).sum(axis=0)
    return out.reshape(B, S, DM)



# revision 16
# speedup vs baseline: 1.2853x; 1.0021x over previous
"""GQA multi-head attention (B=2, S=2048, D=2048, 32 q-heads / 8 kv-heads)
on 8 Trainium2 NeuronCores.

Sharding: tensor-parallel over kv-head groups. Core c owns kv head c and its
4 query heads: Wq column-shard [2048, 256], Wk/Wv column-shard [2048, 64],
Wo row-shard [256, 2048]. Each core computes a full-shape partial output
(its heads' contribution through Wo); the host sums the 8 partials.

Per-core dataflow (all matmuls bf16 operands, fp32 PSUM accumulate). PE cost
on TRN2 is charged per output-free-size row, so every matmul is oriented to
keep the output free dim minimal for the math it does:
  QT  [256, B*S] = Wq_c^T @ q^T     (q^T fed from host; psum [128, 512])
  K   [keys, 64] = kT-tile^T @ Wk_c (input tile stationary: psum [128keys,64],
                                     64-row charge; K^T recovered via DMA XBAR
                                     transpose into ktd, dup'd to both halves)
  V   [keys, 64] = vT-tile^T @ Wv_c (same flip; lands directly in vsb layout)
  S^T [k, q] = KT-tile.T @ QT       (scores transposed: softmax axis on
                                     partitions)
  expS^T = exp(S^T * 1/8)           (ACT, scale folded in; no max-sub:
                                     |scores/8| small so fp32 exp is safe)
  ctx [q, 65] = expS^T-tile.T @ V_aug  (exp tile stationary: 65-row charge;
                                     ones column gives softmax denominator in
                                     free col 64 -> normalization is a fused
                                     per-partition DVE divide in the required
                                     psum->sbuf copy)
  ctx^T via DMA XBAR transpose into ctxT [dq, q]
  out_partial [B*S, 2048] = ctxT-tiles.T @ Wo_c  (psum copies split DVE/Pool)

Schedule: ctx lags scores by one head-step so PE's in-order stream never
waits on ACT exp; batch-1 projections and out-proj groups are interleaved
into batch-0's exp-bound window.
"""
from collections import deque

from contextlib import ExitStack

import numpy as np
import ml_dtypes

import jax

try:
    jax.config.update("jax_compilation_cache_dir", "/tmp/jax_bass_cache")
    jax.config.update("jax_persistent_cache_min_compile_time_secs", 1.0)
except Exception:
    pass

from jax.sharding import Mesh, PartitionSpec, NamedSharding
from jax.experimental.shard_map import shard_map

import concourse.bass as bass
import concourse.mybir as mybir
import concourse.tile as tile
from concourse import bacc, bass2jax

BF16 = mybir.dt.bfloat16
FP8 = mybir.dt.float8e4
F32 = mybir.dt.float32
AF = mybir.ActivationFunctionType

B, S, DM = 2, 2048, 2048
HKV, G, DH = 8, 4, 64
DQ = G * DH            # 256: per-core q-projection width
NC = 8
DT = DM // 128         # 16 contraction tiles
BS = B * S             # 4096
SCALE = 1.0 / 8.0      # 1/sqrt(64)

_cache = {}

import os as _os
# "pe" | "dma": the DMA XBAR route is numerically correct in CoreSim but
# races on the compiled NEFF path, so PE transposes are the default.
TRANS = _os.environ.get("K_TRANS", "pe")


def _emit(ctx, tc, qTh, qTl, kT, vT, wqh, wql, wk, wv, woh, wol, out):
    nc = tc.nc

    pp = ctx.enter_context(tc.tile_pool(name="persist", bufs=1))
    wqh_sb = pp.tile([128, DT, DQ], FP8, tag="wqh")
    wql_sb = pp.tile([128, DT, DQ], FP8, tag="wql")
    wk_sb = pp.tile([128, DT, DH], BF16, tag="wk")
    wv_sb = pp.tile([128, DT, DH], BF16, tag="wv")
    woh_sb = pp.tile([128, 2, DM], FP8, tag="woh")
    wol_sb = pp.tile([128, 2, DM], FP8, tag="wol")
    qtp = pp.tile([128, 2, BS], BF16, tag="qtp")    # QT pairs [p, hp, b*S+s]
    ktd = pp.tile([128, BS], BF16, tag="ktd")       # KT duplicated both halves
    vsb = pp.tile([128, BS // 128, DH + 1], BF16, tag="vsb")  # V + ones col
    ctxTh = pp.tile([128, 2, BS], FP8, tag="ctxTh")  # normalized ctx^T hi
    ctxTl = pp.tile([128, 2, BS], FP8, tag="ctxTl")  # fp8 residual lo

    if TRANS == "pe":
        ident = pp.tile([128, 128], BF16, tag="ident")
        from concourse.masks import make_identity
        make_identity(nc, ident[:])

    # Weight DMA issue order matters: DMA sem-waits hold the SP sequencer,
    # and the lead-in is gated by K/V arrival. wk/wv/wq first (tiny), K/V
    # chunks next, wo deferred past the first q chunk.
    nc.sync.dma_start(wk_sb[:], wk.rearrange("(dt p) m -> p dt m", p=128))
    nc.sync.dma_start(wv_sb[:], wv.rearrange("(dt p) m -> p dt m", p=128))
    nc.sync.dma_start(wqh_sb[:], wqh.rearrange("(dt p) m -> p dt m", p=128))
    nc.sync.dma_start(wql_sb[:], wql.rearrange("(dt p) m -> p dt m", p=128))
    nc.gpsimd.memset(vsb[:, :, DH], 1.0)

    stage = ctx.enter_context(tc.tile_pool(name="stage", bufs=2))
    expp = ctx.enter_context(tc.tile_pool(name="expp", bufs=2))
    smal = ctx.enter_context(tc.tile_pool(name="small", bufs=2))
    outp = ctx.enter_context(tc.tile_pool(name="outp", bufs=2))
    psum = ctx.enter_context(tc.tile_pool(name="psum", bufs=1, space="PSUM"))

    DIV = mybir.AluOpType.divide

    # ---- work-piece pump: ~1-2.5us PE pieces round-robined between the
    # scores pieces of each slot so the 4-deep ACT wait queue never starves.
    work = deque()

    def pump_one():
        while work:
            try:
                next(work[0])
                return True
            except StopIteration:
                work.popleft()
        return False

    def run_gen(g):
        for _ in g:
            pass

    def advance(g):
        return lambda: next(g, None)

    # ---- Phase A emitters -------------------------------------------------
    def a_kv(b, qc):
        """K/V projections, input-tile-stationary: psum [128 keys, 64].
        DMA issue is eager (at call time); compute comes as gen pieces."""
        bo, so = b * S, qc * 512
        k_ch = stage.tile([128, DT, 512], BF16, tag="kv_st", bufs=3,
                          name=f"kch_{b}_{qc}")
        nc.sync.dma_start(
            k_ch[:],
            kT[b].rearrange("(dt p) s -> p dt s", p=128)[:, :, so:so + 512])
        v_ch = stage.tile([128, DT, 512], BF16, tag="kv_st", bufs=3,
                          name=f"vch_{b}_{qc}")
        nc.sync.dma_start(
            v_ch[:],
            vT[b].rearrange("(dt p) s -> p dt s", p=128)[:, :, so:so + 512])

        def gen():
            kv = psum.tile([128, 4, 2, DH], F32, tag="pa", bufs=2,
                           name=f"kv_{b}_{qc}")
            for ki in range(4):
                for dt in range(DT):
                    nc.tensor.matmul(
                        kv[:, ki, 0, :], k_ch[:, dt, ki * 128:(ki + 1) * 128],
                        wk_sb[:, dt, :], start=(dt == 0), stop=(dt == DT - 1))
                for dt in range(DT):
                    nc.tensor.matmul(
                        kv[:, ki, 1, :], v_ch[:, dt, ki * 128:(ki + 1) * 128],
                        wv_sb[:, dt, :], start=(dt == 0), stop=(dt == DT - 1))
                yield
            nc.vector.tensor_copy(
                vsb[:, b * 16 + qc * 4:b * 16 + qc * 4 + 4, 0:DH],
                kv[:, :, 1, :])
            ktmp = smal.tile([128, 4, DH], BF16, tag="ktmp", bufs=2,
                             name=f"ktmp_{b}_{qc}")
            nc.vector.tensor_copy(ktmp[:], kv[:, :, 0, :])
            yield
            if TRANS == "dma":
                for pr in range(2):
                    src = ktmp[:, 2 * pr:2 * pr + 2, :]
                    for half in range(2):
                        dst = ktd[DH * half:DH * half + DH,
                                  bo + so + pr * 256:bo + so + (pr + 1) * 256]
                        nc.sync.dma_start(
                            dst.rearrange("p (t k) -> p t k", t=2), src,
                            transpose=True)
            else:
                for ki in range(4):
                    koff = bo + so + ki * 128
                    ktp = psum.tile([128, 128], BF16, tag="pc", bufs=2,
                                    name=f"ktp_{b}_{qc}_{ki}")
                    for half in range(2):
                        nc.tensor.transpose(
                            ktp[DH * half:DH * half + DH, :], ktmp[:, ki, :],
                            ident[:], tile_position=(0, DH * half))
                    nc.vector.tensor_copy(ktd[:, koff:koff + 128], ktp[:])
            yield

        return gen()

    def a_q_dma(b, qc):
        bo, so = b * S, qc * 512
        qh_ch = stage.tile([128, DT, 512], FP8, tag="q_st", bufs=4,
                           name=f"qhch_{b}_{qc}")
        nc.sync.dma_start(
            qh_ch[:],
            qTh[b].rearrange("(dt p) s -> p dt s", p=128)[:, :, so:so + 512])
        ql_ch = stage.tile([128, DT, 512], FP8, tag="q_st", bufs=4,
                           name=f"qlch_{b}_{qc}")
        nc.sync.dma_start(
            ql_ch[:],
            qTl[b].rearrange("(dt p) s -> p dt s", p=128)[:, :, so:so + 512])
        return (qh_ch, ql_ch)

    def a_q_proj(b, qc, q_ch):
        bo, so = b * S, qc * 512
        qh_ch, ql_ch = q_ch
        terms = [(wqh_sb, qh_ch), (wqh_sb, ql_ch), (wql_sb, qh_ch)]
        for m in range(2):
            pq = psum.tile([128, 512], F32, tag="pa", bufs=2,
                           name=f"pq_{b}_{qc}_{m}")
            n = 0
            for w_sb, qch in terms:
                for t in range(DT // 2):
                    nc.tensor.matmul(
                        pq[:], w_sb[:, 2 * t:2 * t + 2, m * 128:(m + 1) * 128],
                        qch[:, 2 * t:2 * t + 2, :],
                        start=(n == 0), stop=(n == 3 * DT // 2 - 1),
                        perf_mode=mybir.MatmulPerfMode.DoubleRow)
                    n += 1
            nc.vector.tensor_scalar_mul(
                qtp[:, m, bo + so:bo + so + 512], pq[:], 1.0 / 64.0)
            yield

    # ---- Phase C piece ----------------------------------------------------
    def c_st(b, qc, qt):
        st = b * 16 + qc * 4 + qt
        ost = outp.tile([128, DM], BF16, tag="ost", bufs=3, name=f"ost_{st}")
        for ch in range(4):
            po = psum.tile([128, 512], F32, tag="pa", bufs=2,
                           name=f"po_{st}_{ch}")
            for n, (cc, ww) in enumerate(
                    ((ctxTh, woh_sb), (ctxTh, wol_sb), (ctxTl, woh_sb))):
                nc.tensor.matmul(
                    po[:], cc[:, :, st * 128:(st + 1) * 128],
                    ww[:, :, ch * 512:(ch + 1) * 512],
                    start=(n == 0), stop=(n == 2),
                    perf_mode=mybir.MatmulPerfMode.DoubleRow)
            nc.vector.tensor_scalar_mul(
                ost[:, ch * 512:(ch + 1) * 512], po[:], 1.0 / 64.0)
        nc.sync.dma_start(out[st * 128:(st + 1) * 128, :], ost[:])

    # ---- Phase B emitters -------------------------------------------------
    cn_map = {}
    c_backlog = []

    def ctx_gen(b, qc, h, ex, defer_c=False):
        """ctx [q, 65] with exp tile stationary; normalization fused into the
        required psum->sbuf copy; per-qt transposes + out-proj pieces."""
        i, j = h // 2, h % 2
        if j == 0:
            cn_map[(b, qc, i)] = [
                smal.tile([128, 2, DH], BF16, tag="cn", bufs=10,
                          name=f"cn_{b}_{qc}_{i}_{qt}") for qt in range(4)]
        cn = cn_map[(b, qc, i)]
        pcx = psum.tile([128, 4, DH + 1], F32, tag="pc", bufs=2,
                        name=f"pcx_{b}_{qc}_{h}")
        for qt2 in range(2):
            for qt in (2 * qt2, 2 * qt2 + 1):
                for kt in range(DT):
                    nc.tensor.matmul(
                        pcx[:, qt, :], ex[:, kt, qt * 128:(qt + 1) * 128],
                        vsb[:, b * 16 + kt, :],
                        start=(kt == 0), stop=(kt == DT - 1))
            yield
        rr = smal.tile([128, 4], F32, tag="rr", bufs=3, name=f"rr_{b}_{qc}_{h}")
        nc.vector.reciprocal(rr[:], pcx[:, :, DH])
        for qt in range(4):
            nc.vector.tensor_scalar_mul(
                cn[qt][:, j, :], pcx[:, qt, 0:DH], rr[:, qt:qt + 1])
        yield
        if j == 1:
            qoff = b * S + qc * 512
            for qt in range(4):
                dsth = ctxTh[:, i, qoff + qt * 128:qoff + (qt + 1) * 128]
                dstl = ctxTl[:, i, qoff + qt * 128:qoff + (qt + 1) * 128]
                ctp = psum.tile([128, 128], BF16, tag="pc", bufs=2,
                                name=f"ctp_{b}_{qc}_{i}_{qt}")
                nc.tensor.transpose(ctp[:], cn[qt][:], ident[:])
                nc.vector.tensor_copy(dsth, ctp[:])
                nc.vector.tensor_sub(dstl, ctp[:], dsth)
                if i == 1:
                    if defer_c:
                        c_backlog.append((b, qc, qt))
                    else:
                        c_st(b, qc, qt)
                yield
            del cn_map[(b, qc, i)]

    def scores_slot(b, qc, h, actions):
        """Emit the 8 score/exp pieces of slot (qc, h), interleaving one
        action (forced work or deque pump) after each piece."""
        m, j = h // 2, h % 2
        bo = b * S
        qoff = bo + qc * 512
        ex = expp.tile([128, DT, 512], BF16, tag="exp", bufs=2,
                       name=f"ex_{b}_{qc}_{h}")
        for kt2 in range(DT // 2):
            pss = psum.tile([128, 2, 512], F32, tag="sc", bufs=2,
                            name=f"pss_{b}_{qc}_{h}_{kt2}")
            for t in range(2):
                koff = bo + (2 * kt2 + t) * 128
                nc.tensor.matmul(
                    pss[:, t, :], ktd[j * DH:(j + 1) * DH, koff:koff + 128],
                    qtp[j * DH:(j + 1) * DH, m, qoff:qoff + 512])
            nc.scalar.activation(
                ex[:, 2 * kt2:2 * kt2 + 2, :], pss[:], AF.Exp, scale=SCALE)
            # Skip the first two pieces: the previous slot's exp tail is
            # still in flight then, and a pumped ctx piece would stall PE.
            if kt2 >= 2:
                k = kt2 - 2
                if k < len(actions):
                    actions[k]()
                else:
                    pump_one()
        for act in actions[DT // 2 - 2:]:
            act()
        pump_one()
        pump_one()
        return ex

    # ---- Schedule ---------------------------------------------------------
    # Lead-in: K/V chunks stream while projections chase; first q chunk's
    # DMA is slotted before the last K/V pair so q-proj overlaps the tail.
    run_gen(a_kv(0, 0))
    run_gen(a_kv(0, 1))
    run_gen(a_kv(0, 2))
    g_last = a_kv(0, 3)            # issues k3/v3 DMA
    q_ch0 = a_q_dma(0, 0)
    run_gen(g_last)
    run_gen(a_q_proj(0, 0, q_ch0))
    nc.sync.dma_start(woh_sb[:], woh.rearrange("(i p) d -> p i d", p=128))
    nc.sync.dma_start(wol_sb[:], wol.rearrange("(i p) d -> p i d", p=128))

    q_chunks = {}
    q_projs = {}

    for b in range(B):
        pend = deque()
        for qc in range(4):
            for h in range(4):
                actions = []
                if h == 0 and qc < 3:
                    # q chunk (b, qc+1): DMA now, proj pieces forced next slot
                    q_chunks[(b, qc + 1)] = a_q_dma(b, qc + 1)
                    g = a_q_proj(b, qc + 1, q_chunks[(b, qc + 1)])
                    q_projs[(b, qc + 1)] = g
                    actions += [advance(g), advance(g)]
                if b == 0 and h == 1:
                    g = a_kv(1, qc)
                    actions += [advance(g)] * 6
                if b == 0 and qc == 3 and h == 2:
                    q_chunks[(1, 0)] = a_q_dma(1, 0)
                if b == 1 and c_backlog:
                    bb, bqc, bqt = c_backlog.pop(0)
                    actions.append(
                        lambda bb=bb, bqc=bqc, bqt=bqt: c_st(bb, bqc, bqt))
                if pend:
                    prev = pend.popleft()
                    work.append(ctx_gen(b, *prev,
                                        defer_c=(b == 0 and prev[0] >= 2)))
                ex = scores_slot(b, qc, h, actions)
                pend.append((qc, h, ex))
        prev = pend.popleft()
        work.append(ctx_gen(b, *prev, defer_c=(b == 0)))
        if b == 0:
            run_gen(a_q_proj(1, 0, q_chunks[(1, 0)]))
        while pump_one():
            pass


def _build():
    nc = bacc.Bacc("TRN2", target_bir_lowering=False, debug=False, num_devices=NC)
    qTh = nc.dram_tensor("qTh", [B, DM, S], FP8, kind="ExternalInput")
    qTl = nc.dram_tensor("qTl", [B, DM, S], FP8, kind="ExternalInput")
    kT = nc.dram_tensor("kT", [B, DM, S], BF16, kind="ExternalInput")
    vT = nc.dram_tensor("vT", [B, DM, S], BF16, kind="ExternalInput")
    wqh = nc.dram_tensor("wqh", [DM, DQ], FP8, kind="ExternalInput")
    wql = nc.dram_tensor("wql", [DM, DQ], FP8, kind="ExternalInput")
    wk = nc.dram_tensor("wk", [DM, DH], BF16, kind="ExternalInput")
    wv = nc.dram_tensor("wv", [DM, DH], BF16, kind="ExternalInput")
    woh = nc.dram_tensor("woh", [DQ, DM], FP8, kind="ExternalInput")
    wol = nc.dram_tensor("wol", [DQ, DM], FP8, kind="ExternalInput")
    out = nc.dram_tensor("out", [BS, DM], BF16, kind="ExternalOutput")
    with tile.TileContext(nc) as tc:
        with ExitStack() as ctx:
            _emit(ctx, tc, qTh.ap(), qTl.ap(), kT.ap(), vT.ap(), wqh.ap(),
                  wql.ap(), wk.ap(), wv.ap(), woh.ap(), wol.ap(), out.ap())
    nc.compile()
    return nc


def _make_runner(nc, n_cores=NC):
    """Build the sharded jit callable once; reuse across kernel() calls."""
    bass2jax.install_neuronx_cc_hook()
    partition_name = nc.partition_id_tensor.name if nc.partition_id_tensor else None
    in_names, out_names, out_avals, zero_outs = [], [], [], []
    for alloc in nc.m.functions[0].allocations:
        if not isinstance(alloc, mybir.MemoryLocationSet):
            continue
        name = alloc.memorylocations[0].name
        if alloc.kind == "ExternalInput":
            if name != partition_name:
                in_names.append(name)
        elif alloc.kind == "ExternalOutput":
            out_names.append(name)
            shape = tuple(alloc.tensor_shape)
            dtype = mybir.dt.np(alloc.dtype)
            out_avals.append(jax.core.ShapedArray(shape, dtype))
            zero_outs.append(np.zeros(shape, dtype))
    n_params = len(in_names)
    n_outs = len(out_avals)
    in_names_all = in_names + out_names
    if partition_name is not None:
        in_names_all.append(partition_name)
    donate = tuple(range(n_params, n_params + n_outs))

    def _body(*args):
        operands = list(args)
        if partition_name is not None:
            operands.append(bass2jax.partition_id_tensor())
        outs = bass2jax._bass_exec_p.bind(
            *operands,
            out_avals=tuple(out_avals),
            in_names=tuple(in_names_all),
            out_names=tuple(out_names),
            lowering_input_output_aliases=(),
            sim_require_finite=True,
            sim_require_nnan=True,
            nc=nc,
        )
        return tuple(outs)

    devices = jax.devices()[:n_cores]
    mesh = Mesh(np.asarray(devices), ("core",))
    in_specs = (PartitionSpec("core"),) * (n_params + n_outs)
    out_specs = (PartitionSpec("core"),) * len(out_names)
    sharded = jax.jit(
        shard_map(_body, mesh=mesh, in_specs=in_specs, out_specs=out_specs,
                  check_rep=False),
        donate_argnums=donate, keep_unused=True)
    sh = NamedSharding(mesh, PartitionSpec("core"))
    return sharded, in_names, out_names, zero_outs, sh


def _run(in_maps):
    if "nc" not in _cache:
        _cache["nc"] = _build()
    if "runner" not in _cache:
        _cache["runner"] = _make_runner(_cache["nc"])
    sharded, in_names, out_names, zero_outs, sh = _cache["runner"]
    n = NC
    concat_in = [
        jax.device_put(
            np.concatenate([np.asarray(in_maps[c][nm]) for c in range(n)], 0), sh)
        for nm in in_names
    ]
    zeros = [
        jax.device_put(np.zeros((n * z.shape[0], *z.shape[1:]), z.dtype), sh)
        for z in zero_outs
    ]
    outs = sharded(*concat_in, *zeros)
    i = out_names.index("out")
    arr = np.asarray(outs[i])           # [NC*BS, DM]
    return arr.reshape(n, BS, DM)


def kernel(q, k, v, Wq, Wk, Wv, Wo):
    q = np.asarray(q, dtype=np.float32)
    k = np.asarray(k, dtype=np.float32)
    v = np.asarray(v, dtype=np.float32)
    bf = ml_dtypes.bfloat16
    f8 = ml_dtypes.float8_e4m3
    qT32 = np.ascontiguousarray(q.transpose(0, 2, 1))
    qThi = qT32.astype(f8)
    qTlo = (qT32 - qThi.astype(np.float32)).astype(f8)
    kTh = np.ascontiguousarray(k.astype(bf).transpose(0, 2, 1))
    vTh = np.ascontiguousarray(v.astype(bf).transpose(0, 2, 1))
    Wq64 = np.asarray(Wq, dtype=np.float32) * 64.0
    Wqhi = Wq64.astype(f8)
    Wqlo = (Wq64 - Wqhi.astype(np.float32)).astype(f8)
    Wkb = np.asarray(Wk, dtype=np.float32).astype(bf)
    Wvb = np.asarray(Wv, dtype=np.float32).astype(bf)
    Wo64 = np.asarray(Wo, dtype=np.float32) * 64.0
    Wohi = Wo64.astype(f8)
    Wolo = (Wo64 - Wohi.astype(np.float32)).astype(f8)

    in_maps = []
    for c in range(NC):
        in_maps.append({
            "qTh": qThi, "qTl": qTlo, "kT": kTh, "vT": vTh,
            "wqh": np.ascontiguousarray(Wqhi[:, c * DQ:(c + 1) * DQ]),
            "wql": np.ascontiguousarray(Wqlo[:, c * DQ:(c + 1) * DQ]),
            "wk": np.ascontiguousarray(Wkb[:, c * DH:(c + 1) * DH]),
            "wv": np.ascontiguousarray(Wvb[:, c * DH:(c + 1) * DH]),
            "woh": np.ascontiguousarray(Wohi[c * DQ:(c + 1) * DQ, :]),
            "wol": np.ascontiguousarray(Wolo[c * DQ:(c + 1) * DQ, :]),
        })
    partials = _run(in_maps)
    out = partials.astype(np.float32, copy=False).sum(axis=0)
    return out.reshape(B, S, DM)

